# revision 1
# baseline (speedup 1.0000x reference)
"""GAT (graph attention) Bass kernel for TRN2, 8-core SPMD.

Math (exactly equivalent to the reference up to fp reassociation):
  feat = x @ W (per head);  el/er = feat . attn_l/attn_r  ==>  el = x @ wl, er = x @ wr
  g    = feat @ fc_w (per head)                           ==>  g  = x @ WFC
  w_e  = exp(leakyrelu(el[src] + er[dst]))       (softmax without max-subtraction)
  s[d] = sum_{e->d} w_e ;  usum[d] = sum_{e->d} w_e * g[src]
  y[d] = usum[d]/s[d] + bias@fc_w + fc_b

Sharding: dst-range partitioning. Core k owns nodes [k*NPC, (k+1)*NPC).
Each core (replicated) computes the node row table [el|g] for all N nodes,
plus a compact er table for its own range, then gathers rows per edge with
dma_gather (256B rows), and scatter-adds via one-hot matmuls into PSUM
window accumulators (128 dst nodes per window, 11 windows per PSUM bank).

Edges (host-side index prep only) are bucketed by (core, window, src-half)
— the src-half split keeps gather indices < 32768 (int16 limit).
"""

import math
import numpy as np

import concourse.bass as bass
import concourse.mybir as mybir
import concourse.tile as tile
from concourse import bacc, library_config

F32 = mybir.dt.float32
F32R = mybir.dt.float32r
I16 = mybir.dt.int16
I32 = mybir.dt.int32
ALU = mybir.AluOpType
ACTF = mybir.ActivationFunctionType

NEG_SLOPE = 0.2


class Cfg:
    def __init__(self, N=50000, E=1200000, CT=31, GROUPW=11, DMA_SCRATCH=16384):
        self.N = N
        self.E = E
        self.IN = 256           # input feature dim
        self.H = 4              # heads
        self.O = 64             # per-head out dim
        self.D = 10             # final fc dim
        self.NCORES = 8
        self.WIN = 128          # dst nodes per window
        self.GROUPW = GROUPW    # windows per PSUM bank (44*11=484 <= 512 f32)
        self.CT = CT            # tiles per gather call (<= per-call SBUF buf)
        self.DMA_SCRATCH = DMA_SCRATCH
        # single_packet=True caps a call at 64 descs/lane (one packet); we
        # pass single_packet=False. SWDGE ring: CT*8+1 <= 256 descs per lane
        assert CT * 8 + 1 <= 256
        assert N % self.NCORES == 0
        self.NPC = N // self.NCORES
        self.NW = -(-self.NPC // self.WIN)
        self.NG = -(-self.NW // GROUPW)
        assert N % 2 == 0
        self.SPLIT = N // 2
        assert self.SPLIT <= 32767 and self.NPC <= 32767
        self.KI = -(-self.IN // 128)           # i chunks (2)
        self.KHO = -(-(self.H * self.O) // 128)  # ho chunks (2)
        self.AUXW = 2 * self.H + self.H * self.D  # 48: [er|el|g]
        self.ROWW = 64                          # table row width (f32) = 256B
        self.HD = self.H * self.D               # 40
        self.MAINW = self.H + self.HD           # 44: [el|g] in main table


class Structure:
    """Compile-time stream structure shared by host packer and program builder."""

    def __init__(self, cfg: Cfg, T_wh: np.ndarray):
        self.T_wh = T_wh  # [NW, 2] tiles per (window, half)
        tile_meta = []    # (w, half, group_first, group_last)
        call_meta = []    # (tile_start, ntiles, half)
        pos_wh = np.zeros((cfg.NW, 2), np.int64)  # first tile index of run
        for g in range(cfg.NG):
            ws = range(g * cfg.GROUPW, min((g + 1) * cfg.GROUPW, cfg.NW))
            g_first_tile = len(tile_meta)
            g_ntiles = int(T_wh[list(ws), :].sum())
            for half in (0, 1):
                run_start = len(tile_meta)
                for w in ws:
                    pos_wh[w, half] = len(tile_meta)
                    for t in range(T_wh[w, half]):
                        first = (len(tile_meta) == g_first_tile)
                        last = (len(tile_meta) == g_first_tile + g_ntiles - 1)
                        tile_meta.append((w, half, first, last))
                run_len = len(tile_meta) - run_start
                o = 0
                while o < run_len:
                    c = min(cfg.CT, run_len - o)
                    call_meta.append((run_start + o, c, half))
                    o += c
        self.tile_meta = tile_meta
        self.call_meta = call_meta
        self.pos_wh = pos_wh
        self.NT = len(tile_meta)


def preprocess(cfg: Cfg, src: np.ndarray, dst: np.ndarray):
    """Host-side index/layout prep (no float arithmetic).

    Returns (structure, A_idx[NC,128,NT*8] i16, B_idx same, dstoff[NC,128,NT] f32).
    """
    src = np.asarray(src, np.int64)
    dst = np.asarray(dst, np.int64)
    core = dst // cfg.NPC
    dstl = dst - core * cfg.NPC
    w = dstl // cfg.WIN
    off = dstl - w * cfg.WIN
    half = (src >= cfg.SPLIT).astype(np.int64)
    srcr = src - half * cfg.SPLIT

    key = (core * cfg.NW + w) * 2 + half
    order = np.argsort(key, kind="stable")
    nk = cfg.NCORES * cfg.NW * 2
    cnt = np.bincount(key, minlength=nk)
    starts = np.zeros(nk + 1, np.int64)
    np.cumsum(cnt, out=starts[1:])
    cnt_cwh = cnt.reshape(cfg.NCORES, cfg.NW, 2)
    T_wh = np.maximum(-(-cnt_cwh.max(axis=0) // cfg.WIN), 1)  # [NW,2]
    st = Structure(cfg, T_wh)

    NPOS = st.NT * 128
    A_idx = np.zeros((cfg.NCORES, NPOS), np.int16)
    B_idx = np.zeros((cfg.NCORES, NPOS), np.int16)
    dstoff = np.full((cfg.NCORES, NPOS), -1.0, np.float32)
    for c in range(cfg.NCORES):
        for wv in range(cfg.NW):
            for h in (0, 1):
                kk = (c * cfg.NW + wv) * 2 + h
                n = cnt[kk]
                if n == 0:
                    continue
                eids = order[starts[kk]:starts[kk] + n]
                p0 = st.pos_wh[wv, h] * 128
                A_idx[c, p0:p0 + n] = srcr[eids]
                B_idx[c, p0:p0 + n] = dstl[eids]
                dstoff[c, p0:p0 + n] = off[eids]

    def wrap16(a):  # [NPOS] -> [128, NPOS//16]
        return np.tile(np.ascontiguousarray(a.reshape(-1, 16).T), (8, 1))

    def wrap128(a):  # [NPOS] -> [128, NT]
        return np.ascontiguousarray(a.reshape(-1, 128).T)

    A_w = np.stack([wrap16(A_idx[c]) for c in range(cfg.NCORES)])
    B_w = np.stack([wrap16(B_idx[c]) for c in range(cfg.NCORES)])
    D_w = np.stack([wrap128(dstoff[c]) for c in range(cfg.NCORES)])
    return st, A_w, B_w, D_w


def host_layouts(cfg: Cfg, x, W, attn_l, attn_r, bias, fc_w, fc_b):
    """Pure layout transforms of the inputs (no arithmetic)."""
    H, O, D, IN = cfg.H, cfg.O, cfg.D, cfg.IN
    xT = np.ascontiguousarray(np.asarray(x, np.float32).T)          # [IN, N]
    WT = np.ascontiguousarray(
        np.asarray(W, np.float32).transpose(0, 2, 1).reshape(H * O, IN))  # [ho, i]
    wcat = np.zeros((H * O, cfg.AUXW), np.float32)                  # [ho, er|el|g]
    for h in range(H):
        r = slice(h * O, (h + 1) * O)
        wcat[r, h] = attn_r[h]
        wcat[r, H + h] = attn_l[h]
        wcat[r, 2 * H + h * D:2 * H + (h + 1) * D] = fc_w
    bias_flat = np.asarray(bias, np.float32).reshape(H * O, 1)
    fcb_tiled = np.tile(np.asarray(fc_b, np.float32), H).reshape(1, H * D)
    return xT, WT, wcat, bias_flat, fcb_tiled


def build_program(cfg: Cfg, st: Structure):
    nc = bacc.Bacc(trn_type="TRN2", num_swdge_queues=1,
                   dynamic_dma_scratch_size=cfg.DMA_SCRATCH)
    N, IN, H, O, D = cfg.N, cfg.IN, cfg.H, cfg.O, cfg.D
    KI, KHO, AUXW, ROWW, HD, MAINW = (cfg.KI, cfg.KHO, cfg.AUXW, cfg.ROWW,
                                      cfg.HD, cfg.MAINW)
    WIN, NW, NG, GROUPW, NPC, CT = (cfg.WIN, cfg.NW, cfg.NG, cfg.GROUPW,
                                    cfg.NPC, cfg.CT)
    NT = st.NT

    xT = nc.dram_tensor("xT", [IN, N], F32, kind="ExternalInput")
    xTo = nc.dram_tensor("xTown", [IN, NPC], F32, kind="ExternalInput")
    WTt = nc.dram_tensor("WT", [H * O, IN], F32, kind="ExternalInput")
    wcat_t = nc.dram_tensor("wcat", [H * O, AUXW], F32, kind="ExternalInput")
    bias_t = nc.dram_tensor("bias_flat", [H * O, 1], F32, kind="ExternalInput")
    fcb_t = nc.dram_tensor("fcb_tiled", [1, HD], F32, kind="ExternalInput")
    Aidx_t = nc.dram_tensor("A_idx", [128, NT * 8], I16, kind="ExternalInput")
    Bidx_t = nc.dram_tensor("B_idx", [128, NT * 8], I16, kind="ExternalInput")
    doff_t = nc.dram_tensor("dstoff", [128, NT], F32, kind="ExternalInput")
    iota_t = nc.dram_tensor("iota_row", [1, WIN], F32, kind="ExternalInput")
    y_t = nc.dram_tensor("y", [NPC, HD], F32, kind="ExternalOutput")

    row_table = nc.dram_tensor("row_table", [N, ROWW], F32, kind="Internal")
    er_table = nc.dram_tensor("er_table", [NPC, ROWW], F32, kind="Internal")

    NB = 4  # node tiles per phase-1 load batch

    with tile.TileContext(nc) as tc, \
            tc.tile_pool(name="const", bufs=1) as cp, \
            tc.tile_pool(name="p1", bufs=3) as p1, \
            tc.tile_pool(name="p1ps", bufs=2, space="PSUM") as p1ps, \
            tc.tile_pool(name="gath", bufs=4) as gp, \
            tc.tile_pool(name="tp", bufs=6) as tp, \
            tc.tile_pool(name="sp", bufs=10) as sp, \
            tc.tile_pool(name="acc", bufs=3, space="PSUM") as accp, \
            tc.tile_pool(name="outp", bufs=2) as op:

        # ---------- phase 0: constants ----------
        wt_sb = cp.tile([128, KHO, IN], F32)
        wcat_sb = cp.tile([128, KHO, AUXW], F32)
        bf_sb = cp.tile([128, KHO, 1], F32)
        for a in range(KHO):
            r = slice(a * 128, (a + 1) * 128)
            nc.sync.dma_start(out=wt_sb[:, a, :], in_=WTt[r, :])
            nc.sync.dma_start(out=wcat_sb[:, a, :], in_=wcat_t[r, :])
            nc.sync.dma_start(out=bf_sb[:, a, :], in_=bias_t[r, :])
        fcb_sb = cp.tile([1, HD], F32)
        nc.sync.dma_start(out=fcb_sb[:], in_=fcb_t[:])

        aux_sb = cp.tile([128, KI, AUXW], F32)
        for m in range(KI):
            aps = p1ps.tile([128, AUXW], F32, tag="rps")
            for k in range(KHO):
                nc.tensor.matmul(out=aps[:], lhsT=wt_sb[:, k, m * 128:(m + 1) * 128],
                                 rhs=wcat_sb[:, k, :], start=(k == 0), stop=(k == KHO - 1))
            nc.vector.tensor_copy(out=aux_sb[:, m, :], in_=aps[:])

        # bias@fc_w + fc_b, replicated to 128 partitions and GROUPW windows
        brow_ps = p1ps.tile([1, HD], F32, tag="rps")
        for k in range(KHO):
            nc.tensor.matmul(out=brow_ps[:], lhsT=bf_sb[:, k, :],
                             rhs=wcat_sb[:, k, 2 * H:AUXW],
                             start=(k == 0), stop=(k == KHO - 1))
        brow_sb = cp.tile([1, HD], F32)
        nc.vector.tensor_add(out=brow_sb[:], in0=brow_ps[:], in1=fcb_sb[:])
        ones_sb = cp.tile([1, 128], F32)
        nc.vector.memset(ones_sb[:], 1.0)
        brep_ps = p1ps.tile([128, HD], F32, tag="rps")
        nc.tensor.matmul(out=brep_ps[:], lhsT=ones_sb[:], rhs=brow_sb[:],
                         start=True, stop=True)
        brep_sb = cp.tile([128, GROUPW * HD], F32)
        for wl in range(GROUPW):
            nc.vector.tensor_copy(out=brep_sb[:, wl * HD:(wl + 1) * HD], in_=brep_ps[:])

        irow_sb = cp.tile([1, WIN], F32)
        nc.sync.dma_start(out=irow_sb[:], in_=iota_t[:])
        iota_ps = p1ps.tile([128, WIN], F32, tag="rps")
        nc.tensor.matmul(out=iota_ps[:], lhsT=ones_sb[:], rhs=irow_sb[:],
                         start=True, stop=True)
        iota_f = cp.tile([128, WIN], F32)
        nc.vector.tensor_copy(out=iota_f[:], in_=iota_ps[:])

        doff_sb = cp.tile([128, NT], F32)
        nc.sync.dma_start(out=doff_sb[:], in_=doff_t[:])

        # ---------- phase 1: node tables ----------
        def node_pass(src_t, n_nodes, out_tab, cols, tag):
            ncols = cols.stop - cols.start
            ntiles = -(-n_nodes // 128)
            for b in range(0, ntiles, NB):
                bt = min(NB, ntiles - b)
                n0 = b * 128
                bcnt = min(NB * 128, n_nodes - n0)
                xt = p1.tile([128, KI, NB * 128], F32, tag=f"xt{tag}")
                for k in range(KI):
                    nc.sync.dma_start(out=xt[:, k, :bcnt],
                                      in_=src_t[k * 128:(k + 1) * 128, n0:n0 + bcnt])
                rsb = p1.tile([128, NB, ROWW], F32, tag=f"rsb{tag}")
                nc.vector.memset(rsb[:, :, ncols:], 0)
                for j in range(bt):
                    cnt = min(128, n_nodes - (b + j) * 128)
                    rps = p1ps.tile([128, AUXW], F32, tag="rps")
                    for k in range(KI):
                        nc.tensor.matmul(
                            out=rps[:cnt, :ncols],
                            lhsT=xt[:, k, j * 128:j * 128 + cnt],
                            rhs=aux_sb[:, k, cols],
                            start=(k == 0), stop=(k == KI - 1))
                    nc.vector.tensor_copy(out=rsb[:cnt, j, :ncols], in_=rps[:cnt, :ncols])
                if bcnt == bt * 128:
                    out_ap = out_tab[n0:n0 + bt * 128, :].rearrange(
                        "(j p) c -> p j c", p=128)
                    nc.sync.dma_start(out=out_ap, in_=rsb[:, :bt, :])
                else:  # partial final tile: per-tile writes
                    for j in range(bt):
                        cnt = min(128, n_nodes - (b + j) * 128)
                        nc.sync.dma_start(
                            out=out_tab[(b + j) * 128:(b + j) * 128 + cnt, :],
                            in_=rsb[:cnt, j, :])

        # main table rows = [el | g] = aux cols 4:48
        node_pass(xT, N, row_table, slice(H, AUXW), "m")
        # er table rows = [er] = aux cols 0:4, own range only
        node_pass(xTo, NPC, er_table, slice(0, H), "e")

        # ---------- phase 2: edge stream ----------
        cur_g = [-1]
        gps_ref = [None]

        def close_group(g):
            gps = gps_ref[0]
            glen = min(GROUPW, NW - g * GROUPW)
            gv = gps[:].rearrange("p (w c) -> p w c", c=MAINW)
            sg = op.tile([128, GROUPW * H], F32, tag="sg")
            nc.vector.tensor_scalar_max(out=sg[:, :glen * H], in0=gv[:, :glen, 0:H],
                                        scalar1=1e-30)
            rs = op.tile([128, GROUPW * H], F32, tag="rs")
            nc.vector.reciprocal(out=rs[:, :glen * H], in_=sg[:, :glen * H])
            ysb = op.tile([128, GROUPW * HD], F32, tag="ysb")
            nc.vector.tensor_tensor(
                out=ysb[:, :glen * HD].rearrange("p (w h d) -> p w h d", h=H, d=D),
                in0=gv[:, :glen, H:MAINW].rearrange("p w (h d) -> p w h d", h=H),
                in1=rs[:, :glen * H].rearrange("p (w h) -> p w h", h=H)
                    .to_broadcast([128, glen, H, D]),
                op=ALU.mult)
            nc.vector.tensor_add(out=ysb[:, :glen * HD], in0=ysb[:, :glen * HD],
                                 in1=brep_sb[:, :glen * HD])
            for wl in range(glen):
                wv = g * GROUPW + wl
                n0 = wv * WIN
                cnt = min(WIN, NPC - n0)
                nc.sync.dma_start(out=y_t[n0:n0 + cnt, :],
                                  in_=ysb[:cnt, wl * HD:(wl + 1) * HD])

        nreg_cache = {}

        def nreg(n):
            if n not in nreg_cache:
                nreg_cache[n] = nc.gpsimd.to_reg(n)
            return nreg_cache[n]

        for (c0, ctiles, half) in st.call_meta:
            aidx = gp.tile([128, CT * 8], I16, tag="aidx")
            nc.sync.dma_start(out=aidx[:, :ctiles * 8],
                              in_=Aidx_t[:, c0 * 8:(c0 + ctiles) * 8])
            bidx = gp.tile([128, CT * 8], I16, tag="bidx")
            nc.sync.dma_start(out=bidx[:, :ctiles * 8],
                              in_=Bidx_t[:, c0 * 8:(c0 + ctiles) * 8])
            abuf = gp.tile([128, CT, ROWW], F32, tag="abuf")
            tab = row_table[half * cfg.SPLIT:(half + 1) * cfg.SPLIT, :]
            nc.gpsimd.dma_gather(abuf[:, :ctiles, :], tab, aidx[:, :ctiles * 8],
                                 ctiles * 128, nreg(ctiles * 128), ROWW, queue_num=0,
                                 single_packet=False)
            bbuf = gp.tile([128, CT, ROWW], F32, tag="bbuf")
            nc.gpsimd.dma_gather(bbuf[:, :ctiles, :], er_table[:],
                                 bidx[:, :ctiles * 8],
                                 ctiles * 128, nreg(ctiles * 128), ROWW, queue_num=0,
                                 single_packet=False)

            ne = ctiles * H
            esb = tp.tile([128, CT * H], F32, tag="esb")
            nc.vector.tensor_tensor(
                out=esb[:, :ne].rearrange("p (t h) -> p t h", h=H),
                in0=abuf[:, :ctiles, 0:H], in1=bbuf[:, :ctiles, 0:H], op=ALU.add)
            nc.vector.scalar_tensor_tensor(
                out=esb[:, :ne], in0=esb[:, :ne], scalar=NEG_SLOPE,
                in1=esb[:, :ne], op0=ALU.mult, op1=ALU.max)
            # rhs chunk tile: per tile j, cols [0:H]=w, [H:MAINW]=w*g
            mgc = tp.tile([128, CT, MAINW], F32, tag="mgc")
            nc.scalar.activation(out=mgc[:, :ctiles, 0:H], in_=esb[:, :ne],
                                 func=ACTF.Exp)
            nc.vector.tensor_tensor(
                out=mgc[:, :ctiles, H:MAINW].rearrange("p t (h d) -> p t h d", h=H),
                in0=abuf[:, :ctiles, H:MAINW].rearrange("p t (h d) -> p t h d", h=H),
                in1=mgc[:, :ctiles, 0:H].to_broadcast([128, ctiles, H, D]),
                op=ALU.mult)

            for j in range(ctiles):
                tg = c0 + j
                wv, half_, first, last = st.tile_meta[tg]
                g = wv // GROUPW
                if g != cur_g[0]:
                    if cur_g[0] >= 0:
                        close_group(cur_g[0])
                    gps_ref[0] = accp.tile([128, GROUPW * MAINW], F32, tag="gps", name="gps")
                    cur_g[0] = g
                gps = gps_ref[0]
                wloc = wv - g * GROUPW
                S = sp.tile([128, WIN], F32, tag="S")
                nc.vector.tensor_scalar(out=S[:], in0=iota_f[:],
                                        scalar1=doff_sb[:, tg:tg + 1],
                                        scalar2=None, op0=ALU.is_equal)
                base = wloc * MAINW
                nc.tensor.matmul(out=gps[:, base:base + MAINW],
                                 lhsT=S[:], rhs=mgc[:, j, :],
                                 start=first, stop=last)
        close_group(cur_g[0])

    nc.compile()
    return nc


def run_numpy_model(cfg, x, W, attn_l, attn_r, bias, fc_w, fc_b, src, dst):
    """Numpy model of the kernel math (for validation)."""
    feat = np.einsum("ni,hio->nho", x, W)
    el = np.einsum("nho,ho->nh", feat, attn_l)
    er = np.einsum("nho,ho->nh", feat, attn_r)
    e = el[src] + er[dst]
    e = np.where(e > 0, e, NEG_SLOPE * e)
    w = np.exp(e)
    s = np.zeros((cfg.N, cfg.H), np.float32)
    np.add.at(s, dst, w)
    g = np.einsum("nho,od->nhd", feat, fc_w)
    usum = np.zeros((cfg.N, cfg.H, cfg.D), np.float32)
    np.add.at(usum, dst, w[:, :, None] * g[src])
    out = usum / np.maximum(s, 1e-30)[:, :, None]
    return out + (bias @ fc_w)[None] + fc_b[None, None, :]


def make_in_maps(cfg, inputs, A_w, B_w, D_w):
    x = np.asarray(inputs["x"], np.float32)
    xT, WT, wcat, bias_flat, fcb_tiled = host_layouts(
        cfg, x, inputs["W"], inputs["attn_l"], inputs["attn_r"],
        inputs["bias"], inputs["fc_w"], inputs["fc_b"])
    in_maps = []
    for c in range(cfg.NCORES):
        in_maps.append({
            "xT": xT,
            "xTown": np.ascontiguousarray(xT[:, c * cfg.NPC:(c + 1) * cfg.NPC]),
            "WT": WT, "wcat": wcat, "bias_flat": bias_flat,
            "fcb_tiled": fcb_tiled,
            "A_idx": A_w[c], "B_idx": B_w[c], "dstoff": D_w[c],
            "iota_row": np.arange(cfg.WIN, dtype=np.float32).reshape(1, cfg.WIN),
        })
    return in_maps


# ----------------------------------------------------------------------------
# Self-contained entry point: full inputs in, full output out.
# Dst-range edge partitioning across 8 NeuronCores; host side does index /
# layout preparation only, all model arithmetic runs on device.
# ----------------------------------------------------------------------------

def kernel(**inputs):
    import numpy as np
    from concourse import bass_utils

    cfg = Cfg()
    src = np.asarray(inputs["src"])
    dst = np.asarray(inputs["dst"])
    assert src.shape == (cfg.E,) and dst.shape == (cfg.E,)
    st, A_w, B_w, D_w = preprocess(cfg, src, dst)
    nc = build_program(cfg, st)
    in_maps = make_in_maps(cfg, inputs, A_w, B_w, D_w)
    res = bass_utils.run_bass_kernel_spmd(
        nc, in_maps, core_ids=list(range(cfg.NCORES)))
    y = np.concatenate([r["y"] for r in res.results], axis=0)
    return np.ascontiguousarray(y.reshape(cfg.N, cfg.H, cfg.D).astype(np.float32))



# revision 3
# speedup vs baseline: 1.9013x; 1.9013x over previous
"""GAT (graph attention) Bass kernel for TRN2, 8-core SPMD — v2.

Math (equivalent to the reference up to fp reassociation):
  feat = x @ W (per head);  el/er = feat . attn_l/attn_r  ==>  el = x @ wl, er = x @ wr
  g    = feat @ fc_w (per head)                           ==>  g  = x @ WFC
  w_e  = exp(leakyrelu(el[src] + er[dst]))       (softmax without max-subtraction)
  s[d] = sum_{e->d} w_e ;  usum[d] = sum_{e->d} w_e * g[src]
  y[d] = usum[d]/s[d] + bias@fc_w + fc_b

Sharding: dst-range partitioning. Core k owns nodes [k*NPC, (k+1)*NPC).
Each core (replicated) computes the node row table [el|g] for all N nodes
(bf16 matmuls, f32 rows), gathers src rows per edge with dma_gather
(256B rows, the only per-edge DMA), and scatter-adds via one-hot matmuls
into PSUM window accumulators.

v2 vs v1:
  - er[dst] per edge comes from an SBUF-resident er table via a one-hot
    matmul (lhsT=ST) instead of a second dma_gather  -> halves GpSimd time.
  - The per-tile one-hot matrices S (scatter, [edge,slot]) and ST (er
    gather, [slot,edge]) are precomputed on the host as exact bf16 0/1
    matrices and DMA'd in -> removes the per-tile IS_EQ build from DVE.
  - Edge stream is ordered half-0-calls-first so gathers of src-half 0
    overlap the phase-1 build of src-half 1 (row table split into two
    DRAM tensors for independent dependence tracking). All NG=5 group
    accumulators stay open in PSUM simultaneously.
  - Phase-1 projection matmuls run in bf16 (fp32 PE matmuls take 2 passes).

Edges (host-side index prep only) are bucketed by (core, window, src-half)
— the src-half split keeps gather indices < 32768 (int16 limit).
"""

import numpy as np

import concourse.bass as bass
import concourse.mybir as mybir
import concourse.tile as tile
from concourse import bacc

F32 = mybir.dt.float32
BF16 = mybir.dt.bfloat16
I16 = mybir.dt.int16
ALU = mybir.AluOpType
ACTF = mybir.ActivationFunctionType

NEG_SLOPE = 0.2


class Cfg:
    def __init__(self, N=50000, E=1200000, CT=31, GROUPW=11, DMA_SCRATCH=16384):
        self.N = N
        self.E = E
        self.IN = 256           # input feature dim
        self.H = 4              # heads
        self.O = 64             # per-head out dim
        self.D = 10             # final fc dim
        self.NCORES = 8
        self.WIN = 128          # dst nodes per window
        self.GROUPW = GROUPW    # windows per PSUM bank (44*11=484 <= 512 f32)
        self.CT = CT            # tiles per gather call
        self.DMA_SCRATCH = DMA_SCRATCH
        assert CT * 8 + 1 <= 256   # SWDGE ring: descs per lane
        assert N % self.NCORES == 0
        self.NPC = N // self.NCORES
        self.NW = -(-self.NPC // self.WIN)
        self.NG = -(-self.NW // GROUPW)
        assert N % 2 == 0
        self.SPLIT = N // 2
        assert self.SPLIT <= 32767 and self.NPC <= 32767
        self.KI = -(-self.IN // 128)             # x chunks (2)
        self.KHO = -(-(self.H * self.O) // 128)  # ho chunks (2)
        self.AUXW = 2 * self.H + self.H * self.D  # 48: [er|el|g]
        self.ROWW = 64                            # table row width (f32) = 256B
        self.HD = self.H * self.D                 # 40
        self.MAINW = self.H + self.HD             # 44: [el|g] in main table


class Structure:
    """Compile-time stream structure shared by host packer and program builder.

    Stream order is half-major: all half-0 calls (groups 0..NG-1), then all
    half-1 calls.  Each group's PSUM accumulator is opened by its first
    half-0 tile and closed by its last half-1 tile; all NG accumulators are
    live simultaneously.
    """

    def __init__(self, cfg: Cfg, T_wh: np.ndarray):
        self.T_wh = T_wh  # [NW, 2] tiles per (window, half)
        tile_meta = []    # [w, half, first, last]
        call_meta = []    # (tile_start, ntiles, half)
        pos_wh = np.zeros((cfg.NW, 2), np.int64)  # first tile index of bucket
        group_tiles = [[] for _ in range(cfg.NG)]
        for half in (0, 1):
            for g in range(cfg.NG):
                ws = range(g * cfg.GROUPW, min((g + 1) * cfg.GROUPW, cfg.NW))
                run_start = len(tile_meta)
                for w in ws:
                    pos_wh[w, half] = len(tile_meta)
                    for t in range(T_wh[w, half]):
                        group_tiles[g].append(len(tile_meta))
                        tile_meta.append([w, half, False, False])
                run_len = len(tile_meta) - run_start
                o = 0
                while o < run_len:
                    c = min(cfg.CT, run_len - o)
                    call_meta.append((run_start + o, c, half))
                    o += c
        for g in range(cfg.NG):
            tile_meta[group_tiles[g][0]][2] = True    # first (an h0 tile)
            tile_meta[group_tiles[g][-1]][3] = True   # last (an h1 tile)
        self.tile_meta = tile_meta
        self.call_meta = call_meta
        self.pos_wh = pos_wh
        self.NT = len(tile_meta)


def preprocess(cfg: Cfg, src: np.ndarray, dst: np.ndarray):
    """Host-side index/layout prep (no float arithmetic).

    Returns (structure, A_idx[NC,128,NT*8] i16, S[NC,128,NT*128] bf16,
    ST[NC,128,NT*128] bf16).  S/ST are exact 0/1 one-hot matrices.
    """
    import ml_dtypes
    BF = np.dtype(ml_dtypes.bfloat16)

    src = np.asarray(src, np.int64)
    dst = np.asarray(dst, np.int64)
    core = dst // cfg.NPC
    dstl = dst - core * cfg.NPC
    w = dstl // cfg.WIN
    off = dstl - w * cfg.WIN
    half = (src >= cfg.SPLIT).astype(np.int64)
    srcr = src - half * cfg.SPLIT

    key = (core * cfg.NW + w) * 2 + half
    order = np.argsort(key, kind="stable")
    nk = cfg.NCORES * cfg.NW * 2
    cnt = np.bincount(key, minlength=nk)
    starts = np.zeros(nk + 1, np.int64)
    np.cumsum(cnt, out=starts[1:])
    cnt_cwh = cnt.reshape(cfg.NCORES, cfg.NW, 2)
    T_wh = np.maximum(-(-cnt_cwh.max(axis=0) // cfg.WIN), 1)  # [NW,2]
    st = Structure(cfg, T_wh)

    NPOS = st.NT * 128
    A_idx = np.zeros((cfg.NCORES, NPOS), np.int16)
    offp = np.full((cfg.NCORES, NPOS), -1, np.int64)
    for c in range(cfg.NCORES):
        for wv in range(cfg.NW):
            for h in (0, 1):
                kk = (c * cfg.NW + wv) * 2 + h
                n = cnt[kk]
                if n == 0:
                    continue
                eids = order[starts[kk]:starts[kk] + n]
                p0 = st.pos_wh[wv, h] * 128
                A_idx[c, p0:p0 + n] = srcr[eids]
                offp[c, p0:p0 + n] = off[eids]

    def wrap16(a):  # [NPOS] -> [128, NPOS//16]
        return np.tile(np.ascontiguousarray(a.reshape(-1, 16).T), (8, 1))

    A_w = np.stack([wrap16(A_idx[c]) for c in range(cfg.NCORES)])

    pos = np.arange(NPOS, dtype=np.int64)
    S_list, ST_list = [], []
    for c in range(cfg.NCORES):
        o = offp[c]
        valid = o >= 0
        i = pos[valid]
        ov = o[valid]
        S = np.zeros((128, NPOS), BF)
        S[i % 128, (i // 128) * 128 + ov] = 1
        ST = np.zeros((128, NPOS), BF)
        ST[ov, i] = 1
        S_list.append(S)
        ST_list.append(ST)
    return st, A_w, S_list, ST_list


def host_layouts(cfg: Cfg, x, W, attn_l, attn_r, bias, fc_w, fc_b):
    """Pure layout transforms of the inputs (no arithmetic)."""
    H, O, D = cfg.H, cfg.O, cfg.D
    xT = np.ascontiguousarray(np.asarray(x, np.float32).T)          # [IN, N]
    WT = np.ascontiguousarray(
        np.asarray(W, np.float32).transpose(0, 2, 1).reshape(H * O, cfg.IN))
    wcat = np.zeros((H * O, cfg.AUXW), np.float32)                  # [ho, er|el|g]
    for h in range(H):
        r = slice(h * O, (h + 1) * O)
        wcat[r, h] = attn_r[h]
        wcat[r, H + h] = attn_l[h]
        wcat[r, 2 * H + h * D:2 * H + (h + 1) * D] = fc_w
    bias_flat = np.asarray(bias, np.float32).reshape(H * O, 1)
    fcb_tiled = np.tile(np.asarray(fc_b, np.float32), H).reshape(1, H * D)
    return xT, WT, wcat, bias_flat, fcb_tiled


def build_program(cfg: Cfg, st: Structure):
    nc = bacc.Bacc(trn_type="TRN2", num_swdge_queues=1,
                   dynamic_dma_scratch_size=cfg.DMA_SCRATCH)
    N, IN, H, O, D = cfg.N, cfg.IN, cfg.H, cfg.O, cfg.D
    KI, KHO, AUXW, ROWW, HD, MAINW = (cfg.KI, cfg.KHO, cfg.AUXW, cfg.ROWW,
                                      cfg.HD, cfg.MAINW)
    WIN, NW, NG, GROUPW, NPC, CT, SPLIT = (cfg.WIN, cfg.NW, cfg.NG, cfg.GROUPW,
                                           cfg.NPC, cfg.CT, cfg.SPLIT)
    NT = st.NT

    xT = nc.dram_tensor("xT", [IN, N], F32, kind="ExternalInput")
    WTt = nc.dram_tensor("WT", [H * O, IN], F32, kind="ExternalInput")
    wcat_t = nc.dram_tensor("wcat", [H * O, AUXW], F32, kind="ExternalInput")
    bias_t = nc.dram_tensor("bias_flat", [H * O, 1], F32, kind="ExternalInput")
    fcb_t = nc.dram_tensor("fcb_tiled", [1, HD], F32, kind="ExternalInput")
    Aidx_t = nc.dram_tensor("A_idx", [128, NT * 8], I16, kind="ExternalInput")
    S_t = nc.dram_tensor("S_oh", [128, NT * 128], BF16, kind="ExternalInput")
    ST_t = nc.dram_tensor("ST_oh", [128, NT * 128], BF16, kind="ExternalInput")
    y_t = nc.dram_tensor("y", [NPC, HD], F32, kind="ExternalOutput")

    row_h = [nc.dram_tensor(f"row_h{h}", [SPLIT, ROWW], F32, kind="Internal")
             for h in (0, 1)]

    NB = 4  # node tiles per phase-1 load batch
    own0 = None  # filled per-core via own_base input? -- no: SPMD shared program

    # Per-core own range differs between cores, but the program is shared.
    # The er pass reads xT columns [own_base, own_base+NPC); own_base is
    # supplied via a 1-element index DMA... simpler: the er pass uses a
    # dram input holding the own x slice? That re-adds 6.4MB upload.
    # Instead supply own_base as a per-core DRAM slice of xT via a separate
    # ExternalInput xTo view prepared host-side without copying (numpy view).
    xTo = nc.dram_tensor("xTown", [IN, NPC], F32, kind="ExternalInput")

    with tile.TileContext(nc) as tc, \
            tc.tile_pool(name="const", bufs=1) as cp, \
            tc.tile_pool(name="p1", bufs=3) as p1, \
            tc.tile_pool(name="p1ps", bufs=2, space="PSUM") as p1ps, \
            tc.tile_pool(name="gath", bufs=3) as gp, \
            tc.tile_pool(name="tp", bufs=4) as tp, \
            tc.tile_pool(name="erps", bufs=1, space="PSUM") as erp, \
            tc.tile_pool(name="acc", bufs=1, space="PSUM") as accp, \
            tc.tile_pool(name="outp", bufs=2) as op:

        # ---------- phase 0: constants ----------
        wt_sb = cp.tile([128, KHO, IN], F32)
        wcat_sb = cp.tile([128, KHO, AUXW], F32)
        bf_sb = cp.tile([128, KHO, 1], F32)
        for a in range(KHO):
            r = slice(a * 128, (a + 1) * 128)
            nc.sync.dma_start(out=wt_sb[:, a, :], in_=WTt[r, :])
            nc.sync.dma_start(out=wcat_sb[:, a, :], in_=wcat_t[r, :])
            nc.sync.dma_start(out=bf_sb[:, a, :], in_=bias_t[r, :])
        fcb_sb = cp.tile([1, HD], F32)
        nc.sync.dma_start(out=fcb_sb[:], in_=fcb_t[:])

        # aux = WT.T @ wcat : [IN(pad 256), AUXW] ; stored bf16 for phase 1
        auxb = cp.tile([128, KI, AUXW], BF16)
        for m in range(KI):
            aps = p1ps.tile([128, AUXW], F32, tag="rps")
            for k in range(KHO):
                nc.tensor.matmul(out=aps[:], lhsT=wt_sb[:, k, m * 128:(m + 1) * 128],
                                 rhs=wcat_sb[:, k, :], start=(k == 0), stop=(k == KHO - 1))
            nc.vector.tensor_copy(out=auxb[:, m, :], in_=aps[:])

        # bias@fc_w + fc_b, replicated to 128 partitions and GROUPW windows
        brow_ps = p1ps.tile([1, HD], F32, tag="rps")
        for k in range(KHO):
            nc.tensor.matmul(out=brow_ps[:], lhsT=bf_sb[:, k, :],
                             rhs=wcat_sb[:, k, 2 * H:AUXW],
                             start=(k == 0), stop=(k == KHO - 1))
        brow_sb = cp.tile([1, HD], F32)
        nc.vector.tensor_add(out=brow_sb[:], in0=brow_ps[:], in1=fcb_sb[:])
        ones_sb = cp.tile([1, 128], F32)
        nc.vector.memset(ones_sb[:], 1.0)
        brep_ps = p1ps.tile([128, HD], F32, tag="rps")
        nc.tensor.matmul(out=brep_ps[:], lhsT=ones_sb[:], rhs=brow_sb[:],
                         start=True, stop=True)
        brep_sb = cp.tile([128, GROUPW * HD], F32)
        for wl in range(GROUPW):
            nc.vector.tensor_copy(out=brep_sb[:, wl * HD:(wl + 1) * HD], in_=brep_ps[:])

        # ---------- phase 1a: er table for own dst range (SBUF-resident) ----
        er_all = cp.tile([128, NW, H], BF16)
        ntiles_er = -(-NPC // 128)
        for b in range(0, ntiles_er, NB):
            bt = min(NB, ntiles_er - b)
            n0 = b * 128
            bcnt = min(NB * 128, NPC - n0)
            xte = p1.tile([128, KI, NB * 128], F32, tag="xte")
            for k in range(KI):
                nc.sync.dma_start(out=xte[:, k, :bcnt],
                                  in_=xTo[k * 128:(k + 1) * 128, n0:n0 + bcnt])
            xteb = p1.tile([128, KI, NB * 128], BF16, tag="xteb")
            nc.vector.tensor_copy(out=xteb[:, :, :bcnt], in_=xte[:, :, :bcnt])
            for j in range(bt):
                cnt = min(128, NPC - (b + j) * 128)
                rps = p1ps.tile([128, H], F32, tag="rps")
                for k in range(KI):
                    nc.tensor.matmul(
                        out=rps[:cnt, :],
                        lhsT=xteb[:, k, j * 128:j * 128 + cnt],
                        rhs=auxb[:, k, 0:H],
                        start=(k == 0), stop=(k == KI - 1))
                nc.vector.tensor_copy(out=er_all[:cnt, b + j, :], in_=rps[:cnt, :])

        # ---------- phase 1b: main row tables [el|g], one per src half ------
        def half_pass(hf):
            col0 = hf * SPLIT
            ntiles = -(-SPLIT // 128)
            for b in range(0, ntiles, NB):
                bt = min(NB, ntiles - b)
                n0 = b * 128
                bcnt = min(NB * 128, SPLIT - n0)
                xt = p1.tile([128, KI, NB * 128], F32, tag=f"xt{hf}")
                for k in range(KI):
                    nc.sync.dma_start(
                        out=xt[:, k, :bcnt],
                        in_=xT[k * 128:(k + 1) * 128, col0 + n0:col0 + n0 + bcnt])
                xtb = p1.tile([128, KI, NB * 128], BF16, tag=f"xtb{hf}")
                nc.vector.tensor_copy(out=xtb[:, :, :bcnt], in_=xt[:, :, :bcnt])
                rsb = p1.tile([128, NB, ROWW], F32, tag=f"rsb{hf}")
                for j in range(bt):
                    cnt = min(128, SPLIT - (b + j) * 128)
                    rps = p1ps.tile([128, MAINW], F32, tag="rps")
                    for k in range(KI):
                        nc.tensor.matmul(
                            out=rps[:cnt, :],
                            lhsT=xtb[:, k, j * 128:j * 128 + cnt],
                            rhs=auxb[:, k, H:AUXW],
                            start=(k == 0), stop=(k == KI - 1))
                    nc.vector.tensor_copy(out=rsb[:cnt, j, :MAINW], in_=rps[:cnt, :])
                if bcnt == bt * 128:
                    out_ap = row_h[hf][n0:n0 + bt * 128, :].rearrange(
                        "(j p) c -> p j c", p=128)
                    nc.sync.dma_start(out=out_ap, in_=rsb[:, :bt, :])
                else:  # partial final tile: per-tile writes
                    for j in range(bt):
                        cnt = min(128, SPLIT - (b + j) * 128)
                        nc.sync.dma_start(
                            out=row_h[hf][(b + j) * 128:(b + j) * 128 + cnt, :],
                            in_=rsb[:cnt, j, :])

        half_pass(0)
        half_pass(1)

        # ---------- phase 2: edge stream ----------
        gtiles = {}

        def get_gps(g):
            if g not in gtiles:
                gtiles[g] = accp.tile([128, GROUPW * MAINW], F32,
                                      tag=f"gps{g}", name=f"gps{g}")
            return gtiles[g]

        nreg_cache = {}

        def nreg(n):
            if n not in nreg_cache:
                nreg_cache[n] = nc.gpsimd.to_reg(n)
            return nreg_cache[n]

        for (c0, ctiles, half) in st.call_meta:
            aidx = gp.tile([128, CT * 8], I16, tag="aidx")
            nc.sync.dma_start(out=aidx[:, :ctiles * 8],
                              in_=Aidx_t[:, c0 * 8:(c0 + ctiles) * 8])
            S_sb = gp.tile([128, CT, WIN], BF16, tag="S_sb")
            nc.sync.dma_start(out=S_sb[:, :ctiles, :],
                              in_=S_t[:, c0 * 128:(c0 + ctiles) * 128])
            ST_sb = gp.tile([128, CT, WIN], BF16, tag="ST_sb")
            nc.sync.dma_start(out=ST_sb[:, :ctiles, :],
                              in_=ST_t[:, c0 * 128:(c0 + ctiles) * 128])
            abuf = gp.tile([128, CT, ROWW], F32, tag="abuf")
            nc.gpsimd.dma_gather(abuf[:, :ctiles, :], row_h[half][:, :],
                                 aidx[:, :ctiles * 8],
                                 ctiles * 128, nreg(ctiles * 128), ROWW, queue_num=0,
                                 single_packet=False)

            # er per edge: one-hot gather matmul from SBUF er table
            er_ps = erp.tile([128, CT * H], F32, tag="erps")
            for j in range(ctiles):
                wv = st.tile_meta[c0 + j][0]
                nc.tensor.matmul(out=er_ps[:, j * H:(j + 1) * H],
                                 lhsT=ST_sb[:, j, :], rhs=er_all[:, wv, :],
                                 start=True, stop=True)

            ne = ctiles * H
            esb = tp.tile([128, CT * H], F32, tag="esb")
            nc.vector.tensor_tensor(
                out=esb[:, :ne].rearrange("p (t h) -> p t h", h=H),
                in0=abuf[:, :ctiles, 0:H],
                in1=er_ps[:, :ne].rearrange("p (t h) -> p t h", h=H),
                op=ALU.add)
            nc.vector.scalar_tensor_tensor(
                out=esb[:, :ne], in0=esb[:, :ne], scalar=NEG_SLOPE,
                in1=esb[:, :ne], op0=ALU.mult, op1=ALU.max)
            nc.scalar.activation(out=esb[:, :ne], in_=esb[:, :ne], func=ACTF.Exp)
            # rhs chunk tile: per tile j, cols [0:H]=w (bf16), [H:MAINW]=w*g
            mgc = tp.tile([128, CT, MAINW], BF16, tag="mgc")
            nc.vector.tensor_copy(
                out=mgc[:, :ctiles, 0:H], in_=esb[:, :ne].rearrange(
                    "p (t h) -> p t h", h=H))
            nc.vector.tensor_tensor(
                out=mgc[:, :ctiles, H:MAINW].rearrange("p t (h d) -> p t h d", h=H),
                in0=abuf[:, :ctiles, H:MAINW].rearrange("p t (h d) -> p t h d", h=H),
                in1=esb[:, :ne].rearrange("p (t h) -> p t h", h=H)
                    .to_broadcast([128, ctiles, H, D]),
                op=ALU.mult)

            for j in range(ctiles):
                wv, half_, first, last = st.tile_meta[c0 + j]
                g = wv // GROUPW
                gps = get_gps(g)
                wloc = wv - g * GROUPW
                base = wloc * MAINW
                nc.tensor.matmul(out=gps[:, base:base + MAINW],
                                 lhsT=S_sb[:, j, :], rhs=mgc[:, j, :],
                                 start=first, stop=last)

        # ---------- phase 3: normalize + output ----------
        for g in range(NG):
            gps = gtiles[g]
            glen = min(GROUPW, NW - g * GROUPW)
            gv = gps[:].rearrange("p (w c) -> p w c", c=MAINW)
            sg = op.tile([128, GROUPW * H], F32, tag="sg")
            nc.vector.tensor_scalar_max(out=sg[:, :glen * H], in0=gv[:, :glen, 0:H],
                                        scalar1=1e-30)
            rs = op.tile([128, GROUPW * H], F32, tag="rs")
            nc.vector.reciprocal(out=rs[:, :glen * H], in_=sg[:, :glen * H])
            ysb = op.tile([128, GROUPW * HD], F32, tag="ysb")
            nc.vector.tensor_tensor(
                out=ysb[:, :glen * HD].rearrange("p (w h d) -> p w h d", h=H, d=D),
                in0=gv[:, :glen, H:MAINW].rearrange("p w (h d) -> p w h d", h=H),
                in1=rs[:, :glen * H].rearrange("p (w h) -> p w h", h=H)
                    .to_broadcast([128, glen, H, D]),
                op=ALU.mult)
            nc.vector.tensor_add(out=ysb[:, :glen * HD], in0=ysb[:, :glen * HD],
                                 in1=brep_sb[:, :glen * HD])
            for wl in range(glen):
                wv = g * GROUPW + wl
                n0 = wv * WIN
                cnt = min(WIN, NPC - n0)
                nc.sync.dma_start(out=y_t[n0:n0 + cnt, :],
                                  in_=ysb[:cnt, wl * HD:(wl + 1) * HD])

    nc.compile()
    return nc


def run_numpy_model(cfg, x, W, attn_l, attn_r, bias, fc_w, fc_b, src, dst):
    """Numpy model of the kernel math (for validation)."""
    feat = np.einsum("ni,hio->nho", x, W)
    el = np.einsum("nho,ho->nh", feat, attn_l)
    er = np.einsum("nho,ho->nh", feat, attn_r)
    e = el[src] + er[dst]
    e = np.where(e > 0, e, NEG_SLOPE * e)
    w = np.exp(e)
    s = np.zeros((cfg.N, cfg.H), np.float32)
    np.add.at(s, dst, w)
    g = np.einsum("nho,od->nhd", feat, fc_w)
    usum = np.zeros((cfg.N, cfg.H, cfg.D), np.float32)
    np.add.at(usum, dst, w[:, :, None] * g[src])
    out = usum / np.maximum(s, 1e-30)[:, :, None]
    return out + (bias @ fc_w)[None] + fc_b[None, None, :]


def make_in_maps(cfg, inputs, A_w, S_list, ST_list):
    x = np.asarray(inputs["x"], np.float32)
    xT, WT, wcat, bias_flat, fcb_tiled = host_layouts(
        cfg, x, inputs["W"], inputs["attn_l"], inputs["attn_r"],
        inputs["bias"], inputs["fc_w"], inputs["fc_b"])
    in_maps = []
    for c in range(cfg.NCORES):
        in_maps.append({
            "xT": xT,
            "xTown": np.ascontiguousarray(xT[:, c * cfg.NPC:(c + 1) * cfg.NPC]),
            "WT": WT, "wcat": wcat, "bias_flat": bias_flat,
            "fcb_tiled": fcb_tiled,
            "A_idx": A_w[c], "S_oh": S_list[c], "ST_oh": ST_list[c],
        })
    return in_maps


# ----------------------------------------------------------------------------
# Self-contained entry point: full inputs in, full output out.
# ----------------------------------------------------------------------------

def kernel(**inputs):
    import numpy as np
    from concourse import bass_utils

    cfg = Cfg()
    src = np.asarray(inputs["src"])
    dst = np.asarray(inputs["dst"])
    assert src.shape == (cfg.E,) and dst.shape == (cfg.E,)
    st, A_w, S_list, ST_list = preprocess(cfg, src, dst)
    nc = build_program(cfg, st)
    in_maps = make_in_maps(cfg, inputs, A_w, S_list, ST_list)
    res = bass_utils.run_bass_kernel_spmd(
        nc, in_maps, core_ids=list(range(cfg.NCORES)))
    y = np.concatenate([r["y"] for r in res.results], axis=0)
    return np.ascontiguousarray(y.reshape(cfg.N, cfg.H, cfg.D).astype(np.float32))


# revision 4
# speedup vs baseline: 2.0011x; 1.0525x over previous
"""GAT (graph attention) Bass kernel for TRN2, 8-core SPMD — v2.

Math (equivalent to the reference up to fp reassociation):
  feat = x @ W (per head);  el/er = feat . attn_l/attn_r  ==>  el = x @ wl, er = x @ wr
  g    = feat @ fc_w (per head)                           ==>  g  = x @ WFC
  w_e  = exp(leakyrelu(el[src] + er[dst]))       (softmax without max-subtraction)
  s[d] = sum_{e->d} w_e ;  usum[d] = sum_{e->d} w_e * g[src]
  y[d] = usum[d]/s[d] + bias@fc_w + fc_b

Sharding: dst-range partitioning. Core k owns nodes [k*NPC, (k+1)*NPC).
Each core (replicated) computes the node row table [el|g] for all N nodes
(bf16 matmuls, f32 rows), gathers src rows per edge with dma_gather
(256B rows, the only per-edge DMA), and scatter-adds via one-hot matmuls
into PSUM window accumulators.

v2 vs v1:
  - er[dst] per edge comes from an SBUF-resident er table via a one-hot
    matmul (lhsT=ST) instead of a second dma_gather  -> halves GpSimd time.
  - The per-tile one-hot matrices S (scatter, [edge,slot]) and ST (er
    gather, [slot,edge]) are precomputed on the host as exact bf16 0/1
    matrices and DMA'd in -> removes the per-tile IS_EQ build from DVE.
  - Edge stream is ordered half-0-calls-first so gathers of src-half 0
    overlap the phase-1 build of src-half 1 (row table split into two
    DRAM tensors for independent dependence tracking). All NG=5 group
    accumulators stay open in PSUM simultaneously.
  - Phase-1 projection matmuls run in bf16 (fp32 PE matmuls take 2 passes).

Edges (host-side index prep only) are bucketed by (core, window, src-half)
— the src-half split keeps gather indices < 32768 (int16 limit).
"""

import numpy as np

import concourse.bass as bass
import concourse.mybir as mybir
import concourse.tile as tile
from concourse import bacc

F32 = mybir.dt.float32
BF16 = mybir.dt.bfloat16
I16 = mybir.dt.int16
ALU = mybir.AluOpType
ACTF = mybir.ActivationFunctionType

NEG_SLOPE = 0.2


class Cfg:
    def __init__(self, N=50000, E=1200000, CT=31, GROUPW=11, DMA_SCRATCH=16384):
        self.N = N
        self.E = E
        self.IN = 256           # input feature dim
        self.H = 4              # heads
        self.O = 64             # per-head out dim
        self.D = 10             # final fc dim
        self.NCORES = 8
        self.WIN = 128          # dst nodes per window
        self.GROUPW = GROUPW    # windows per PSUM bank (44*11=484 <= 512 f32)
        self.CT = CT            # tiles per gather call
        self.DMA_SCRATCH = DMA_SCRATCH
        assert CT * 8 + 1 <= 256   # SWDGE ring: descs per lane
        assert N % self.NCORES == 0
        self.NPC = N // self.NCORES
        self.NW = -(-self.NPC // self.WIN)
        self.NG = -(-self.NW // GROUPW)
        assert N % 2 == 0
        self.SPLIT = N // 2
        assert self.SPLIT <= 32767 and self.NPC <= 32767
        self.KI = -(-self.IN // 128)             # x chunks (2)
        self.KHO = -(-(self.H * self.O) // 128)  # ho chunks (2)
        self.AUXW = 2 * self.H + self.H * self.D  # 48: [er|el|g]
        self.ROWW = 64                            # table row width (f32) = 256B
        self.HD = self.H * self.D                 # 40
        self.MAINW = self.H + self.HD             # 44: [el|g] in main table


class Structure:
    """Compile-time stream structure shared by host packer and program builder.

    Stream order is half-major: all half-0 calls (groups 0..NG-1), then all
    half-1 calls.  Each group's PSUM accumulator is opened by its first
    half-0 tile and closed by its last half-1 tile; all NG accumulators are
    live simultaneously.
    """

    def __init__(self, cfg: Cfg, T_wh: np.ndarray):
        self.T_wh = T_wh  # [NW, 2] tiles per (window, half)
        tile_meta = []    # [w, half, first, last]
        call_meta = []    # (tile_start, ntiles, half)
        pos_wh = np.zeros((cfg.NW, 2), np.int64)  # first tile index of bucket
        group_tiles = [[] for _ in range(cfg.NG)]
        for half in (0, 1):
            for g in range(cfg.NG):
                ws = range(g * cfg.GROUPW, min((g + 1) * cfg.GROUPW, cfg.NW))
                run_start = len(tile_meta)
                for w in ws:
                    pos_wh[w, half] = len(tile_meta)
                    for t in range(T_wh[w, half]):
                        group_tiles[g].append(len(tile_meta))
                        tile_meta.append([w, half, False, False])
                run_len = len(tile_meta) - run_start
                o = 0
                while o < run_len:
                    c = min(cfg.CT, run_len - o)
                    call_meta.append((run_start + o, c, half))
                    o += c
        for g in range(cfg.NG):
            tile_meta[group_tiles[g][0]][2] = True    # first (an h0 tile)
            tile_meta[group_tiles[g][-1]][3] = True   # last (an h1 tile)
        self.tile_meta = tile_meta
        self.call_meta = call_meta
        self.pos_wh = pos_wh
        self.NT = len(tile_meta)


def preprocess(cfg: Cfg, src: np.ndarray, dst: np.ndarray):
    """Host-side index/layout prep (no float arithmetic).

    Returns (structure, A_idx[NC,128,NT*8] i16, S[NC,128,NT*128] bf16,
    ST[NC,128,NT*128] bf16).  S/ST are exact 0/1 one-hot matrices.
    """
    import ml_dtypes
    BF = np.dtype(ml_dtypes.bfloat16)

    src = np.asarray(src, np.int64)
    dst = np.asarray(dst, np.int64)
    core = dst // cfg.NPC
    dstl = dst - core * cfg.NPC
    w = dstl // cfg.WIN
    off = dstl - w * cfg.WIN
    half = (src >= cfg.SPLIT).astype(np.int64)
    srcr = src - half * cfg.SPLIT

    key = (core * cfg.NW + w) * 2 + half
    order = np.argsort(key, kind="stable")
    nk = cfg.NCORES * cfg.NW * 2
    cnt = np.bincount(key, minlength=nk)
    starts = np.zeros(nk + 1, np.int64)
    np.cumsum(cnt, out=starts[1:])
    cnt_cwh = cnt.reshape(cfg.NCORES, cfg.NW, 2)
    T_wh = np.maximum(-(-cnt_cwh.max(axis=0) // cfg.WIN), 1)  # [NW,2]
    st = Structure(cfg, T_wh)

    NPOS = st.NT * 128
    A_idx = np.zeros((cfg.NCORES, NPOS), np.int16)
    offp = np.full((cfg.NCORES, NPOS), -1, np.int64)
    for c in range(cfg.NCORES):
        for wv in range(cfg.NW):
            for h in (0, 1):
                kk = (c * cfg.NW + wv) * 2 + h
                n = cnt[kk]
                if n == 0:
                    continue
                eids = order[starts[kk]:starts[kk] + n]
                p0 = st.pos_wh[wv, h] * 128
                A_idx[c, p0:p0 + n] = srcr[eids]
                offp[c, p0:p0 + n] = off[eids]

    def wrap16(a):  # [NPOS] -> [128, NPOS//16]
        return np.tile(np.ascontiguousarray(a.reshape(-1, 16).T), (8, 1))

    A_w = np.stack([wrap16(A_idx[c]) for c in range(cfg.NCORES)])

    pos = np.arange(NPOS, dtype=np.int64)
    S_list, ST_list = [], []
    for c in range(cfg.NCORES):
        o = offp[c]
        valid = o >= 0
        i = pos[valid]
        ov = o[valid]
        S = np.zeros((128, NPOS), BF)
        S[i % 128, (i // 128) * 128 + ov] = 1
        ST = np.zeros((128, NPOS), BF)
        ST[ov, i] = 1
        S_list.append(S)
        ST_list.append(ST)
    return st, A_w, S_list, ST_list


def host_layouts(cfg: Cfg, x, W, attn_l, attn_r, bias, fc_w, fc_b):
    """Pure layout transforms of the inputs (no arithmetic)."""
    H, O, D = cfg.H, cfg.O, cfg.D
    xT = np.ascontiguousarray(np.asarray(x, np.float32).T)          # [IN, N]
    WT = np.ascontiguousarray(
        np.asarray(W, np.float32).transpose(0, 2, 1).reshape(H * O, cfg.IN))
    wcat = np.zeros((H * O, cfg.AUXW), np.float32)                  # [ho, er|el|g]
    for h in range(H):
        r = slice(h * O, (h + 1) * O)
        wcat[r, h] = attn_r[h]
        wcat[r, H + h] = attn_l[h]
        wcat[r, 2 * H + h * D:2 * H + (h + 1) * D] = fc_w
    bias_flat = np.asarray(bias, np.float32).reshape(H * O, 1)
    fcb_tiled = np.tile(np.asarray(fc_b, np.float32), H).reshape(1, H * D)
    return xT, WT, wcat, bias_flat, fcb_tiled


def build_program(cfg: Cfg, st: Structure):
    nc = bacc.Bacc(trn_type="TRN2", num_swdge_queues=1,
                   dynamic_dma_scratch_size=cfg.DMA_SCRATCH)
    N, IN, H, O, D = cfg.N, cfg.IN, cfg.H, cfg.O, cfg.D
    KI, KHO, AUXW, ROWW, HD, MAINW = (cfg.KI, cfg.KHO, cfg.AUXW, cfg.ROWW,
                                      cfg.HD, cfg.MAINW)
    WIN, NW, NG, GROUPW, NPC, CT, SPLIT = (cfg.WIN, cfg.NW, cfg.NG, cfg.GROUPW,
                                           cfg.NPC, cfg.CT, cfg.SPLIT)
    NT = st.NT

    xT = nc.dram_tensor("xT", [IN, N], BF16, kind="ExternalInput")
    WTt = nc.dram_tensor("WT", [H * O, IN], F32, kind="ExternalInput")
    wcat_t = nc.dram_tensor("wcat", [H * O, AUXW], F32, kind="ExternalInput")
    bias_t = nc.dram_tensor("bias_flat", [H * O, 1], F32, kind="ExternalInput")
    fcb_t = nc.dram_tensor("fcb_tiled", [1, HD], F32, kind="ExternalInput")
    Aidx_t = nc.dram_tensor("A_idx", [128, NT * 8], I16, kind="ExternalInput")
    S_t = nc.dram_tensor("S_oh", [128, NT * 128], BF16, kind="ExternalInput")
    ST_t = nc.dram_tensor("ST_oh", [128, NT * 128], BF16, kind="ExternalInput")
    y_t = nc.dram_tensor("y", [NPC, HD], F32, kind="ExternalOutput")

    row_h = [nc.dram_tensor(f"row_h{h}", [SPLIT, ROWW], F32, kind="Internal")
             for h in (0, 1)]

    NB = 4  # node tiles per phase-1 load batch
    own0 = None  # filled per-core via own_base input? -- no: SPMD shared program

    # Per-core own range differs between cores, but the program is shared.
    # The er pass reads xT columns [own_base, own_base+NPC); own_base is
    # supplied via a 1-element index DMA... simpler: the er pass uses a
    # dram input holding the own x slice? That re-adds 6.4MB upload.
    # Instead supply own_base as a per-core DRAM slice of xT via a separate
    # ExternalInput xTo view prepared host-side without copying (numpy view).
    xTo = nc.dram_tensor("xTown", [IN, NPC], BF16, kind="ExternalInput")

    with tile.TileContext(nc) as tc, \
            tc.tile_pool(name="const", bufs=1) as cp, \
            tc.tile_pool(name="p1", bufs=3) as p1, \
            tc.tile_pool(name="p1ps", bufs=2, space="PSUM") as p1ps, \
            tc.tile_pool(name="gath", bufs=3) as gp, \
            tc.tile_pool(name="tp", bufs=4) as tp, \
            tc.tile_pool(name="erps", bufs=1, space="PSUM") as erp, \
            tc.tile_pool(name="acc", bufs=1, space="PSUM") as accp, \
            tc.tile_pool(name="outp", bufs=2) as op:

        # ---------- phase 0: constants ----------
        wt_sb = cp.tile([128, KHO, IN], F32)
        wcat_sb = cp.tile([128, KHO, AUXW], F32)
        bf_sb = cp.tile([128, KHO, 1], F32)
        for a in range(KHO):
            r = slice(a * 128, (a + 1) * 128)
            nc.sync.dma_start(out=wt_sb[:, a, :], in_=WTt[r, :])
            nc.sync.dma_start(out=wcat_sb[:, a, :], in_=wcat_t[r, :])
            nc.sync.dma_start(out=bf_sb[:, a, :], in_=bias_t[r, :])
        fcb_sb = cp.tile([1, HD], F32)
        nc.sync.dma_start(out=fcb_sb[:], in_=fcb_t[:])

        # aux = WT.T @ wcat : [IN(pad 256), AUXW] ; stored bf16 for phase 1
        auxb = cp.tile([128, KI, AUXW], BF16)
        for m in range(KI):
            aps = p1ps.tile([128, AUXW], F32, tag="rps")
            for k in range(KHO):
                nc.tensor.matmul(out=aps[:], lhsT=wt_sb[:, k, m * 128:(m + 1) * 128],
                                 rhs=wcat_sb[:, k, :], start=(k == 0), stop=(k == KHO - 1))
            nc.vector.tensor_copy(out=auxb[:, m, :], in_=aps[:])

        # bias@fc_w + fc_b, replicated to 128 partitions and GROUPW windows
        brow_ps = p1ps.tile([1, HD], F32, tag="rps")
        for k in range(KHO):
            nc.tensor.matmul(out=brow_ps[:], lhsT=bf_sb[:, k, :],
                             rhs=wcat_sb[:, k, 2 * H:AUXW],
                             start=(k == 0), stop=(k == KHO - 1))
        brow_sb = cp.tile([1, HD], F32)
        nc.vector.tensor_add(out=brow_sb[:], in0=brow_ps[:], in1=fcb_sb[:])
        ones_sb = cp.tile([1, 128], F32)
        nc.vector.memset(ones_sb[:], 1.0)
        brep_ps = p1ps.tile([128, HD], F32, tag="rps")
        nc.tensor.matmul(out=brep_ps[:], lhsT=ones_sb[:], rhs=brow_sb[:],
                         start=True, stop=True)
        brep_sb = cp.tile([128, GROUPW * HD], F32)
        for wl in range(GROUPW):
            nc.vector.tensor_copy(out=brep_sb[:, wl * HD:(wl + 1) * HD], in_=brep_ps[:])

        # ---------- phase 1a: er table for own dst range (SBUF-resident) ----
        er_all = cp.tile([128, NW, H], BF16)

        def er_pass():
            ntiles_er = -(-NPC // 128)
            for b in range(0, ntiles_er, NB):
                bt = min(NB, ntiles_er - b)
                n0 = b * 128
                bcnt = min(NB * 128, NPC - n0)
                xte = p1.tile([128, KI, NB * 128], BF16, tag="xte")
                for k in range(KI):
                    nc.sync.dma_start(out=xte[:, k, :bcnt],
                                      in_=xTo[k * 128:(k + 1) * 128, n0:n0 + bcnt])
                for j in range(bt):
                    cnt = min(128, NPC - (b + j) * 128)
                    rps = p1ps.tile([128, H], F32, tag="rps")
                    for k in range(KI):
                        nc.tensor.matmul(
                            out=rps[:cnt, :],
                            lhsT=xte[:, k, j * 128:j * 128 + cnt],
                            rhs=auxb[:, k, 0:H],
                            start=(k == 0), stop=(k == KI - 1))
                    nc.vector.tensor_copy(out=er_all[:cnt, b + j, :], in_=rps[:cnt, :])

        # ---------- phase 1b: main row tables [el|g], one per src half ------
        def half_pass(hf):
            col0 = hf * SPLIT
            ntiles = -(-SPLIT // 128)
            for b in range(0, ntiles, NB):
                bt = min(NB, ntiles - b)
                n0 = b * 128
                bcnt = min(NB * 128, SPLIT - n0)
                xt = p1.tile([128, KI, NB * 128], BF16, tag=f"xt{hf}")
                for k in range(KI):
                    nc.sync.dma_start(
                        out=xt[:, k, :bcnt],
                        in_=xT[k * 128:(k + 1) * 128, col0 + n0:col0 + n0 + bcnt])
                rsb = p1.tile([128, NB, ROWW], F32, tag=f"rsb{hf}")
                for j in range(bt):
                    cnt = min(128, SPLIT - (b + j) * 128)
                    rps = p1ps.tile([128, MAINW], F32, tag="rps")
                    for k in range(KI):
                        nc.tensor.matmul(
                            out=rps[:cnt, :],
                            lhsT=xt[:, k, j * 128:j * 128 + cnt],
                            rhs=auxb[:, k, H:AUXW],
                            start=(k == 0), stop=(k == KI - 1))
                    nc.vector.tensor_copy(out=rsb[:cnt, j, :MAINW], in_=rps[:cnt, :])
                if bcnt == bt * 128:
                    out_ap = row_h[hf][n0:n0 + bt * 128, :].rearrange(
                        "(j p) c -> p j c", p=128)
                    nc.sync.dma_start(out=out_ap, in_=rsb[:, :bt, :])
                else:  # partial final tile: per-tile writes
                    for j in range(bt):
                        cnt = min(128, SPLIT - (b + j) * 128)
                        nc.sync.dma_start(
                            out=row_h[hf][(b + j) * 128:(b + j) * 128 + cnt, :],
                            in_=rsb[:cnt, j, :])

        half_pass(0)
        er_pass()
        half_pass(1)

        # ---------- phase 2: edge stream ----------
        gtiles = {}

        def get_gps(g):
            if g not in gtiles:
                gtiles[g] = accp.tile([128, GROUPW * MAINW], F32,
                                      tag=f"gps{g}", name=f"gps{g}")
            return gtiles[g]

        nreg_cache = {}

        def nreg(n):
            if n not in nreg_cache:
                nreg_cache[n] = nc.gpsimd.to_reg(n)
            return nreg_cache[n]

        for (c0, ctiles, half) in st.call_meta:
            aidx = gp.tile([128, CT * 8], I16, tag="aidx")
            nc.sync.dma_start(out=aidx[:, :ctiles * 8],
                              in_=Aidx_t[:, c0 * 8:(c0 + ctiles) * 8])
            S_sb = gp.tile([128, CT, WIN], BF16, tag="S_sb")
            nc.sync.dma_start(out=S_sb[:, :ctiles, :],
                              in_=S_t[:, c0 * 128:(c0 + ctiles) * 128])
            ST_sb = gp.tile([128, CT, WIN], BF16, tag="ST_sb")
            nc.sync.dma_start(out=ST_sb[:, :ctiles, :],
                              in_=ST_t[:, c0 * 128:(c0 + ctiles) * 128])
            abuf = gp.tile([128, CT, ROWW], F32, tag="abuf")
            nc.gpsimd.dma_gather(abuf[:, :ctiles, :], row_h[half][:, :],
                                 aidx[:, :ctiles * 8],
                                 ctiles * 128, nreg(ctiles * 128), ROWW, queue_num=0,
                                 single_packet=False)

            # er per edge: one-hot gather matmul from SBUF er table
            er_ps = erp.tile([128, CT * H], F32, tag="erps")
            for j in range(ctiles):
                wv = st.tile_meta[c0 + j][0]
                nc.tensor.matmul(out=er_ps[:, j * H:(j + 1) * H],
                                 lhsT=ST_sb[:, j, :], rhs=er_all[:, wv, :],
                                 start=True, stop=True)

            ne = ctiles * H
            esb = tp.tile([128, CT * H], F32, tag="esb")
            nc.vector.tensor_tensor(
                out=esb[:, :ne].rearrange("p (t h) -> p t h", h=H),
                in0=abuf[:, :ctiles, 0:H],
                in1=er_ps[:, :ne].rearrange("p (t h) -> p t h", h=H),
                op=ALU.add)
            nc.vector.scalar_tensor_tensor(
                out=esb[:, :ne], in0=esb[:, :ne], scalar=NEG_SLOPE,
                in1=esb[:, :ne], op0=ALU.mult, op1=ALU.max)
            nc.scalar.activation(out=esb[:, :ne], in_=esb[:, :ne], func=ACTF.Exp)
            # rhs chunk tile: per tile j, cols [0:H]=w (bf16), [H:MAINW]=w*g
            mgc = tp.tile([128, CT, MAINW], BF16, tag="mgc")
            nc.vector.tensor_copy(
                out=mgc[:, :ctiles, 0:H], in_=esb[:, :ne].rearrange(
                    "p (t h) -> p t h", h=H))
            nc.vector.tensor_tensor(
                out=mgc[:, :ctiles, H:MAINW].rearrange("p t (h d) -> p t h d", h=H),
                in0=abuf[:, :ctiles, H:MAINW].rearrange("p t (h d) -> p t h d", h=H),
                in1=esb[:, :ne].rearrange("p (t h) -> p t h", h=H)
                    .to_broadcast([128, ctiles, H, D]),
                op=ALU.mult)

            for j in range(ctiles):
                wv, half_, first, last = st.tile_meta[c0 + j]
                g = wv // GROUPW
                gps = get_gps(g)
                wloc = wv - g * GROUPW
                base = wloc * MAINW
                nc.tensor.matmul(out=gps[:, base:base + MAINW],
                                 lhsT=S_sb[:, j, :], rhs=mgc[:, j, :],
                                 start=first, stop=last)

        # ---------- phase 3: normalize + output ----------
        for g in range(NG):
            gps = gtiles[g]
            glen = min(GROUPW, NW - g * GROUPW)
            gv = gps[:].rearrange("p (w c) -> p w c", c=MAINW)
            sg = op.tile([128, GROUPW * H], F32, tag="sg")
            nc.vector.tensor_scalar_max(out=sg[:, :glen * H], in0=gv[:, :glen, 0:H],
                                        scalar1=1e-30)
            rs = op.tile([128, GROUPW * H], F32, tag="rs")
            nc.vector.reciprocal(out=rs[:, :glen * H], in_=sg[:, :glen * H])
            ysb = op.tile([128, GROUPW * HD], F32, tag="ysb")
            nc.vector.tensor_tensor(
                out=ysb[:, :glen * HD].rearrange("p (w h d) -> p w h d", h=H, d=D),
                in0=gv[:, :glen, H:MAINW].rearrange("p w (h d) -> p w h d", h=H),
                in1=rs[:, :glen * H].rearrange("p (w h) -> p w h", h=H)
                    .to_broadcast([128, glen, H, D]),
                op=ALU.mult)
            nc.vector.tensor_add(out=ysb[:, :glen * HD], in0=ysb[:, :glen * HD],
                                 in1=brep_sb[:, :glen * HD])
            for wl in range(glen):
                wv = g * GROUPW + wl
                n0 = wv * WIN
                cnt = min(WIN, NPC - n0)
                nc.sync.dma_start(out=y_t[n0:n0 + cnt, :],
                                  in_=ysb[:cnt, wl * HD:(wl + 1) * HD])

    nc.compile()
    return nc


def run_numpy_model(cfg, x, W, attn_l, attn_r, bias, fc_w, fc_b, src, dst):
    """Numpy model of the kernel math (for validation)."""
    feat = np.einsum("ni,hio->nho", x, W)
    el = np.einsum("nho,ho->nh", feat, attn_l)
    er = np.einsum("nho,ho->nh", feat, attn_r)
    e = el[src] + er[dst]
    e = np.where(e > 0, e, NEG_SLOPE * e)
    w = np.exp(e)
    s = np.zeros((cfg.N, cfg.H), np.float32)
    np.add.at(s, dst, w)
    g = np.einsum("nho,od->nhd", feat, fc_w)
    usum = np.zeros((cfg.N, cfg.H, cfg.D), np.float32)
    np.add.at(usum, dst, w[:, :, None] * g[src])
    out = usum / np.maximum(s, 1e-30)[:, :, None]
    return out + (bias @ fc_w)[None] + fc_b[None, None, :]


def make_in_maps(cfg, inputs, A_w, S_list, ST_list):
    import ml_dtypes
    BF = np.dtype(ml_dtypes.bfloat16)
    x = np.asarray(inputs["x"], np.float32)
    xT, WT, wcat, bias_flat, fcb_tiled = host_layouts(
        cfg, x, inputs["W"], inputs["attn_l"], inputs["attn_r"],
        inputs["bias"], inputs["fc_w"], inputs["fc_b"])
    xTb = np.ascontiguousarray(xT.astype(BF))
    in_maps = []
    for c in range(cfg.NCORES):
        in_maps.append({
            "xT": xTb,
            "xTown": np.ascontiguousarray(xTb[:, c * cfg.NPC:(c + 1) * cfg.NPC]),
            "WT": WT, "wcat": wcat, "bias_flat": bias_flat,
            "fcb_tiled": fcb_tiled,
            "A_idx": A_w[c], "S_oh": S_list[c], "ST_oh": ST_list[c],
        })
    return in_maps


# ----------------------------------------------------------------------------
# Self-contained entry point: full inputs in, full output out.
# ----------------------------------------------------------------------------

def kernel(**inputs):
    import numpy as np
    from concourse import bass_utils

    cfg = Cfg()
    src = np.asarray(inputs["src"])
    dst = np.asarray(inputs["dst"])
    assert src.shape == (cfg.E,) and dst.shape == (cfg.E,)
    st, A_w, S_list, ST_list = preprocess(cfg, src, dst)
    nc = build_program(cfg, st)
    in_maps = make_in_maps(cfg, inputs, A_w, S_list, ST_list)
    res = bass_utils.run_bass_kernel_spmd(
        nc, in_maps, core_ids=list(range(cfg.NCORES)))
    y = np.concatenate([r["y"] for r in res.results], axis=0)
    return np.ascontiguousarray(y.reshape(cfg.N, cfg.H, cfg.D).astype(np.float32))


# revision 5
# speedup vs baseline: 2.0198x; 1.0093x over previous
"""GAT (graph attention) Bass kernel for TRN2, 8-core SPMD — v2.

Math (equivalent to the reference up to fp reassociation):
  feat = x @ W (per head);  el/er = feat . attn_l/attn_r  ==>  el = x @ wl, er = x @ wr
  g    = feat @ fc_w (per head)                           ==>  g  = x @ WFC
  w_e  = exp(leakyrelu(el[src] + er[dst]))       (softmax without max-subtraction)
  s[d] = sum_{e->d} w_e ;  usum[d] = sum_{e->d} w_e * g[src]
  y[d] = usum[d]/s[d] + bias@fc_w + fc_b

Sharding: dst-range partitioning. Core k owns nodes [k*NPC, (k+1)*NPC).
Each core (replicated) computes the node row table [el|g] for all N nodes
(bf16 matmuls, f32 rows), gathers src rows per edge with dma_gather
(256B rows, the only per-edge DMA), and scatter-adds via one-hot matmuls
into PSUM window accumulators.

v2 vs v1:
  - er[dst] per edge comes from an SBUF-resident er table via a one-hot
    matmul (lhsT=ST) instead of a second dma_gather  -> halves GpSimd time.
  - The per-tile one-hot matrices S (scatter, [edge,slot]) and ST (er
    gather, [slot,edge]) are precomputed on the host as exact bf16 0/1
    matrices and DMA'd in -> removes the per-tile IS_EQ build from DVE.
  - Edge stream is ordered half-0-calls-first so gathers of src-half 0
    overlap the phase-1 build of src-half 1 (row table split into two
    DRAM tensors for independent dependence tracking). All NG=5 group
    accumulators stay open in PSUM simultaneously.
  - Phase-1 projection matmuls run in bf16 (fp32 PE matmuls take 2 passes).

Edges (host-side index prep only) are bucketed by (core, window, src-half)
— the src-half split keeps gather indices < 32768 (int16 limit).
"""

import numpy as np

import concourse.bass as bass
import concourse.mybir as mybir
import concourse.tile as tile
from concourse import bacc

F32 = mybir.dt.float32
BF16 = mybir.dt.bfloat16
FP8 = mybir.dt.float8e4
I16 = mybir.dt.int16
ALU = mybir.AluOpType
ACTF = mybir.ActivationFunctionType

NEG_SLOPE = 0.2


class Cfg:
    def __init__(self, N=50000, E=1200000, CT=31, GROUPW=11, DMA_SCRATCH=16384):
        self.N = N
        self.E = E
        self.IN = 256           # input feature dim
        self.H = 4              # heads
        self.O = 64             # per-head out dim
        self.D = 10             # final fc dim
        self.NCORES = 8
        self.WIN = 128          # dst nodes per window
        self.GROUPW = GROUPW    # windows per PSUM bank (44*11=484 <= 512 f32)
        self.CT = CT            # tiles per gather call
        self.DMA_SCRATCH = DMA_SCRATCH
        assert CT * 8 + 1 <= 256   # SWDGE ring: descs per lane
        assert N % self.NCORES == 0
        self.NPC = N // self.NCORES
        self.NW = -(-self.NPC // self.WIN)
        self.NG = -(-self.NW // GROUPW)
        assert N % 2 == 0
        self.SPLIT = N // 2
        assert self.SPLIT <= 32767 and self.NPC <= 32767
        self.KI = -(-self.IN // 128)             # x chunks (2)
        self.KHO = -(-(self.H * self.O) // 128)  # ho chunks (2)
        self.AUXW = 2 * self.H + self.H * self.D  # 48: [er|el|g]
        self.ROWW = 64                            # table row width (f32) = 256B
        self.HD = self.H * self.D                 # 40
        self.MAINW = self.H + self.HD             # 44: [el|g] in main table


class Structure:
    """Compile-time stream structure shared by host packer and program builder.

    Stream order is half-major: all half-0 calls (groups 0..NG-1), then all
    half-1 calls.  Each group's PSUM accumulator is opened by its first
    half-0 tile and closed by its last half-1 tile; all NG accumulators are
    live simultaneously.
    """

    def __init__(self, cfg: Cfg, T_wh: np.ndarray):
        self.T_wh = T_wh  # [NW, 2] tiles per (window, half)
        tile_meta = []    # [w, half, first, last]
        call_meta = []    # (tile_start, ntiles, half)
        pos_wh = np.zeros((cfg.NW, 2), np.int64)  # first tile index of bucket
        group_tiles = [[] for _ in range(cfg.NG)]
        for half in (0, 1):
            for g in range(cfg.NG):
                ws = range(g * cfg.GROUPW, min((g + 1) * cfg.GROUPW, cfg.NW))
                run_start = len(tile_meta)
                for w in ws:
                    pos_wh[w, half] = len(tile_meta)
                    for t in range(T_wh[w, half]):
                        group_tiles[g].append(len(tile_meta))
                        tile_meta.append([w, half, False, False])
                run_len = len(tile_meta) - run_start
                o = 0
                while o < run_len:
                    c = min(cfg.CT, run_len - o)
                    call_meta.append((run_start + o, c, half))
                    o += c
        for g in range(cfg.NG):
            tile_meta[group_tiles[g][0]][2] = True    # first (an h0 tile)
            tile_meta[group_tiles[g][-1]][3] = True   # last (an h1 tile)
        c0, ct, hf = call_meta[-1]
        if ct > 6:
            call_meta[-1] = (c0, ct - 4, hf)
            call_meta.append((c0 + ct - 4, 4, hf))
        self.tile_meta = tile_meta
        self.call_meta = call_meta
        self.pos_wh = pos_wh
        self.NT = len(tile_meta)


def preprocess(cfg: Cfg, src: np.ndarray, dst: np.ndarray):
    """Host-side index/layout prep (no float arithmetic).

    Returns (structure, A_idx[NC,128,NT*8] i16, S[NC,128,NT*128] bf16,
    ST[NC,128,NT*128] bf16).  S/ST are exact 0/1 one-hot matrices.
    """
    OH_DT = np.dtype(mybir.dt.np(FP8))

    src = np.asarray(src, np.int64)
    dst = np.asarray(dst, np.int64)
    core = dst // cfg.NPC
    dstl = dst - core * cfg.NPC
    w = dstl // cfg.WIN
    off = dstl - w * cfg.WIN
    half = (src >= cfg.SPLIT).astype(np.int64)
    srcr = src - half * cfg.SPLIT

    key = (core * cfg.NW + w) * 2 + half
    order = np.argsort(key, kind="stable")
    nk = cfg.NCORES * cfg.NW * 2
    cnt = np.bincount(key, minlength=nk)
    starts = np.zeros(nk + 1, np.int64)
    np.cumsum(cnt, out=starts[1:])
    cnt_cwh = cnt.reshape(cfg.NCORES, cfg.NW, 2)
    T_wh = np.maximum(-(-cnt_cwh.max(axis=0) // cfg.WIN), 1)  # [NW,2]
    st = Structure(cfg, T_wh)

    NPOS = st.NT * 128
    A_idx = np.zeros((cfg.NCORES, NPOS), np.int16)
    offp = np.full((cfg.NCORES, NPOS), -1, np.int64)
    for c in range(cfg.NCORES):
        for wv in range(cfg.NW):
            for h in (0, 1):
                kk = (c * cfg.NW + wv) * 2 + h
                n = cnt[kk]
                if n == 0:
                    continue
                eids = order[starts[kk]:starts[kk] + n]
                p0 = st.pos_wh[wv, h] * 128
                A_idx[c, p0:p0 + n] = srcr[eids]
                offp[c, p0:p0 + n] = off[eids]

    def wrap16(a):  # [NPOS] -> [128, NPOS//16]
        return np.tile(np.ascontiguousarray(a.reshape(-1, 16).T), (8, 1))

    A_w = np.stack([wrap16(A_idx[c]) for c in range(cfg.NCORES)])

    pos = np.arange(NPOS, dtype=np.int64)
    S_list, ST_list = [], []
    for c in range(cfg.NCORES):
        o = offp[c]
        valid = o >= 0
        i = pos[valid]
        ov = o[valid]
        S = np.zeros((128, NPOS), OH_DT)
        S[i % 128, (i // 128) * 128 + ov] = 1
        ST = np.zeros((128, NPOS), OH_DT)
        ST[ov, i] = 1
        S_list.append(S)
        ST_list.append(ST)
    return st, A_w, S_list, ST_list


def host_layouts(cfg: Cfg, x, W, attn_l, attn_r, bias, fc_w, fc_b):
    """Pure layout transforms of the inputs (no arithmetic)."""
    H, O, D = cfg.H, cfg.O, cfg.D
    xT = np.ascontiguousarray(np.asarray(x, np.float32).T)          # [IN, N]
    WT = np.ascontiguousarray(
        np.asarray(W, np.float32).transpose(0, 2, 1).reshape(H * O, cfg.IN))
    wcat = np.zeros((H * O, cfg.AUXW), np.float32)                  # [ho, er|el|g]
    for h in range(H):
        r = slice(h * O, (h + 1) * O)
        wcat[r, h] = attn_r[h]
        wcat[r, H + h] = attn_l[h]
        wcat[r, 2 * H + h * D:2 * H + (h + 1) * D] = fc_w
    bias_flat = np.asarray(bias, np.float32).reshape(H * O, 1)
    fcb_tiled = np.tile(np.asarray(fc_b, np.float32), H).reshape(1, H * D)
    return xT, WT, wcat, bias_flat, fcb_tiled


def build_program(cfg: Cfg, st: Structure):
    nc = bacc.Bacc(trn_type="TRN2", num_swdge_queues=1,
                   dynamic_dma_scratch_size=cfg.DMA_SCRATCH)
    N, IN, H, O, D = cfg.N, cfg.IN, cfg.H, cfg.O, cfg.D
    KI, KHO, AUXW, ROWW, HD, MAINW = (cfg.KI, cfg.KHO, cfg.AUXW, cfg.ROWW,
                                      cfg.HD, cfg.MAINW)
    WIN, NW, NG, GROUPW, NPC, CT, SPLIT = (cfg.WIN, cfg.NW, cfg.NG, cfg.GROUPW,
                                           cfg.NPC, cfg.CT, cfg.SPLIT)
    NT = st.NT

    xT = nc.dram_tensor("xT", [IN, N], BF16, kind="ExternalInput")
    WTt = nc.dram_tensor("WT", [H * O, IN], F32, kind="ExternalInput")
    wcat_t = nc.dram_tensor("wcat", [H * O, AUXW], F32, kind="ExternalInput")
    bias_t = nc.dram_tensor("bias_flat", [H * O, 1], F32, kind="ExternalInput")
    fcb_t = nc.dram_tensor("fcb_tiled", [1, HD], F32, kind="ExternalInput")
    Aidx_t = nc.dram_tensor("A_idx", [128, NT * 8], I16, kind="ExternalInput")
    S_t = nc.dram_tensor("S_oh", [128, NT * 128], FP8, kind="ExternalInput")
    ST_t = nc.dram_tensor("ST_oh", [128, NT * 128], FP8, kind="ExternalInput")
    y_t = nc.dram_tensor("y", [NPC, HD], F32, kind="ExternalOutput")

    row_h = [nc.dram_tensor(f"row_h{h}", [SPLIT, ROWW], F32, kind="Internal")
             for h in (0, 1)]

    NB = 4  # node tiles per phase-1 load batch
    own0 = None  # filled per-core via own_base input? -- no: SPMD shared program

    # Per-core own range differs between cores, but the program is shared.
    # The er pass reads xT columns [own_base, own_base+NPC); own_base is
    # supplied via a 1-element index DMA... simpler: the er pass uses a
    # dram input holding the own x slice? That re-adds 6.4MB upload.
    # Instead supply own_base as a per-core DRAM slice of xT via a separate
    # ExternalInput xTo view prepared host-side without copying (numpy view).
    xTo = nc.dram_tensor("xTown", [IN, NPC], BF16, kind="ExternalInput")

    with tile.TileContext(nc) as tc, \
            tc.tile_pool(name="const", bufs=1) as cp, \
            tc.tile_pool(name="p1", bufs=3) as p1, \
            tc.tile_pool(name="p1ps", bufs=2, space="PSUM") as p1ps, \
            tc.tile_pool(name="gath", bufs=3) as gp, \
            tc.tile_pool(name="tp", bufs=4) as tp, \
            tc.tile_pool(name="erps", bufs=1, space="PSUM") as erp, \
            tc.tile_pool(name="acc", bufs=1, space="PSUM") as accp, \
            tc.tile_pool(name="outp", bufs=2) as op:

        # ---------- phase 0: constants ----------
        wt_sb = cp.tile([128, KHO, IN], F32)
        wcat_sb = cp.tile([128, KHO, AUXW], F32)
        bf_sb = cp.tile([128, KHO, 1], F32)
        for a in range(KHO):
            r = slice(a * 128, (a + 1) * 128)
            nc.sync.dma_start(out=wt_sb[:, a, :], in_=WTt[r, :])
            nc.sync.dma_start(out=wcat_sb[:, a, :], in_=wcat_t[r, :])
            nc.sync.dma_start(out=bf_sb[:, a, :], in_=bias_t[r, :])
        fcb_sb = cp.tile([1, HD], F32)
        nc.sync.dma_start(out=fcb_sb[:], in_=fcb_t[:])

        # aux = WT.T @ wcat : [IN(pad 256), AUXW] ; stored bf16 for phase 1
        auxb = cp.tile([128, KI, AUXW], BF16)
        for m in range(KI):
            aps = p1ps.tile([128, AUXW], F32, tag="rps")
            for k in range(KHO):
                nc.tensor.matmul(out=aps[:], lhsT=wt_sb[:, k, m * 128:(m + 1) * 128],
                                 rhs=wcat_sb[:, k, :], start=(k == 0), stop=(k == KHO - 1))
            nc.vector.tensor_copy(out=auxb[:, m, :], in_=aps[:])

        # bias@fc_w + fc_b, replicated to 128 partitions and GROUPW windows
        brow_ps = p1ps.tile([1, HD], F32, tag="rps")
        for k in range(KHO):
            nc.tensor.matmul(out=brow_ps[:], lhsT=bf_sb[:, k, :],
                             rhs=wcat_sb[:, k, 2 * H:AUXW],
                             start=(k == 0), stop=(k == KHO - 1))
        brow_sb = cp.tile([1, HD], F32)
        nc.vector.tensor_add(out=brow_sb[:], in0=brow_ps[:], in1=fcb_sb[:])
        ones_sb = cp.tile([1, 128], F32)
        nc.vector.memset(ones_sb[:], 1.0)
        brep_ps = p1ps.tile([128, HD], F32, tag="rps")
        nc.tensor.matmul(out=brep_ps[:], lhsT=ones_sb[:], rhs=brow_sb[:],
                         start=True, stop=True)
        brep_sb = cp.tile([128, GROUPW * HD], F32)
        for wl in range(GROUPW):
            nc.vector.tensor_copy(out=brep_sb[:, wl * HD:(wl + 1) * HD], in_=brep_ps[:])

        # ---------- phase 1a: er table for own dst range (SBUF-resident) ----
        er_all = cp.tile([128, NW, H], BF16)

        def er_pass():
            ntiles_er = -(-NPC // 128)
            for b in range(0, ntiles_er, NB):
                bt = min(NB, ntiles_er - b)
                n0 = b * 128
                bcnt = min(NB * 128, NPC - n0)
                xte = p1.tile([128, KI, NB * 128], BF16, tag="xte")
                for k in range(KI):
                    nc.sync.dma_start(out=xte[:, k, :bcnt],
                                      in_=xTo[k * 128:(k + 1) * 128, n0:n0 + bcnt])
                for j in range(bt):
                    cnt = min(128, NPC - (b + j) * 128)
                    rps = p1ps.tile([128, H], F32, tag="rps")
                    for k in range(KI):
                        nc.tensor.matmul(
                            out=rps[:cnt, :],
                            lhsT=xte[:, k, j * 128:j * 128 + cnt],
                            rhs=auxb[:, k, 0:H],
                            start=(k == 0), stop=(k == KI - 1))
                    nc.vector.tensor_copy(out=er_all[:cnt, b + j, :], in_=rps[:cnt, :])

        # ---------- phase 1b: main row tables [el|g], one per src half ------
        def half_pass(hf):
            col0 = hf * SPLIT
            ntiles = -(-SPLIT // 128)
            for b in range(0, ntiles, NB):
                bt = min(NB, ntiles - b)
                n0 = b * 128
                bcnt = min(NB * 128, SPLIT - n0)
                xt = p1.tile([128, KI, NB * 128], BF16, tag=f"xt{hf}")
                for k in range(KI):
                    nc.sync.dma_start(
                        out=xt[:, k, :bcnt],
                        in_=xT[k * 128:(k + 1) * 128, col0 + n0:col0 + n0 + bcnt])
                rsb = p1.tile([128, NB, ROWW], F32, tag=f"rsb{hf}")
                for j in range(bt):
                    cnt = min(128, SPLIT - (b + j) * 128)
                    rps = p1ps.tile([128, MAINW], F32, tag="rps")
                    for k in range(KI):
                        nc.tensor.matmul(
                            out=rps[:cnt, :],
                            lhsT=xt[:, k, j * 128:j * 128 + cnt],
                            rhs=auxb[:, k, H:AUXW],
                            start=(k == 0), stop=(k == KI - 1))
                    nc.vector.tensor_copy(out=rsb[:cnt, j, :MAINW], in_=rps[:cnt, :])
                if bcnt == bt * 128:
                    out_ap = row_h[hf][n0:n0 + bt * 128, :].rearrange(
                        "(j p) c -> p j c", p=128)
                    nc.sync.dma_start(out=out_ap, in_=rsb[:, :bt, :])
                else:  # partial final tile: per-tile writes
                    for j in range(bt):
                        cnt = min(128, SPLIT - (b + j) * 128)
                        nc.sync.dma_start(
                            out=row_h[hf][(b + j) * 128:(b + j) * 128 + cnt, :],
                            in_=rsb[:cnt, j, :])

        er_pass()
        half_pass(0)
        half_pass(1)

        # ---------- phase 2: edge stream ----------
        gtiles = {}

        def get_gps(g):
            if g not in gtiles:
                gtiles[g] = accp.tile([128, GROUPW * MAINW], F32,
                                      tag=f"gps{g}", name=f"gps{g}")
            return gtiles[g]

        nreg_cache = {}

        def nreg(n):
            if n not in nreg_cache:
                nreg_cache[n] = nc.gpsimd.to_reg(n)
            return nreg_cache[n]

        for (c0, ctiles, half) in st.call_meta:
            aidx = gp.tile([128, CT * 8], I16, tag="aidx", bufs=4)
            nc.sync.dma_start(out=aidx[:, :ctiles * 8],
                              in_=Aidx_t[:, c0 * 8:(c0 + ctiles) * 8])
            S_sb = gp.tile([128, CT, WIN], FP8, tag="S_sb", bufs=4)
            nc.sync.dma_start(out=S_sb[:, :ctiles, :],
                              in_=S_t[:, c0 * 128:(c0 + ctiles) * 128])
            ST_sb = gp.tile([128, CT, WIN], FP8, tag="ST_sb", bufs=4)
            nc.sync.dma_start(out=ST_sb[:, :ctiles, :],
                              in_=ST_t[:, c0 * 128:(c0 + ctiles) * 128])
            abuf = gp.tile([128, CT, ROWW], F32, tag="abuf", bufs=4)
            nc.gpsimd.dma_gather(abuf[:, :ctiles, :], row_h[half][:, :],
                                 aidx[:, :ctiles * 8],
                                 ctiles * 128, nreg(ctiles * 128), ROWW, queue_num=0,
                                 single_packet=False)

            # er per edge: one-hot gather matmul from SBUF er table
            er_ps = erp.tile([128, CT * H], F32, tag="erps")
            for j in range(ctiles):
                wv = st.tile_meta[c0 + j][0]
                nc.tensor.matmul(out=er_ps[:, j * H:(j + 1) * H],
                                 lhsT=ST_sb[:, j, :], rhs=er_all[:, wv, :],
                                 start=True, stop=True)

            ne = ctiles * H
            esb = tp.tile([128, CT * H], F32, tag="esb")
            nc.vector.tensor_tensor(
                out=esb[:, :ne].rearrange("p (t h) -> p t h", h=H),
                in0=abuf[:, :ctiles, 0:H],
                in1=er_ps[:, :ne].rearrange("p (t h) -> p t h", h=H),
                op=ALU.add)
            nc.vector.scalar_tensor_tensor(
                out=esb[:, :ne], in0=esb[:, :ne], scalar=NEG_SLOPE,
                in1=esb[:, :ne], op0=ALU.mult, op1=ALU.max)
            nc.scalar.activation(out=esb[:, :ne], in_=esb[:, :ne], func=ACTF.Exp)
            # rhs chunk tile: per tile j, cols [0:H]=w (bf16), [H:MAINW]=w*g
            mgc = tp.tile([128, CT, MAINW], BF16, tag="mgc")
            nc.vector.tensor_copy(
                out=mgc[:, :ctiles, 0:H], in_=esb[:, :ne].rearrange(
                    "p (t h) -> p t h", h=H))
            nc.vector.tensor_tensor(
                out=mgc[:, :ctiles, H:MAINW].rearrange("p t (h d) -> p t h d", h=H),
                in0=abuf[:, :ctiles, H:MAINW].rearrange("p t (h d) -> p t h d", h=H),
                in1=esb[:, :ne].rearrange("p (t h) -> p t h", h=H)
                    .to_broadcast([128, ctiles, H, D]),
                op=ALU.mult)

            for j in range(ctiles):
                wv, half_, first, last = st.tile_meta[c0 + j]
                g = wv // GROUPW
                gps = get_gps(g)
                wloc = wv - g * GROUPW
                base = wloc * MAINW
                nc.tensor.matmul(out=gps[:, base:base + MAINW],
                                 lhsT=S_sb[:, j, :], rhs=mgc[:, j, :],
                                 start=first, stop=last)

        # ---------- phase 3: normalize + output ----------
        for g in range(NG):
            gps = gtiles[g]
            glen = min(GROUPW, NW - g * GROUPW)
            gv = gps[:].rearrange("p (w c) -> p w c", c=MAINW)
            sg = op.tile([128, GROUPW * H], F32, tag="sg")
            nc.vector.tensor_scalar_max(out=sg[:, :glen * H], in0=gv[:, :glen, 0:H],
                                        scalar1=1e-30)
            rs = op.tile([128, GROUPW * H], F32, tag="rs")
            nc.vector.reciprocal(out=rs[:, :glen * H], in_=sg[:, :glen * H])
            ysb = op.tile([128, GROUPW * HD], F32, tag="ysb")
            nc.vector.tensor_tensor(
                out=ysb[:, :glen * HD].rearrange("p (w h d) -> p w h d", h=H, d=D),
                in0=gv[:, :glen, H:MAINW].rearrange("p w (h d) -> p w h d", h=H),
                in1=rs[:, :glen * H].rearrange("p (w h) -> p w h", h=H)
                    .to_broadcast([128, glen, H, D]),
                op=ALU.mult)
            nc.vector.tensor_add(out=ysb[:, :glen * HD], in0=ysb[:, :glen * HD],
                                 in1=brep_sb[:, :glen * HD])
            for wl in range(glen):
                wv = g * GROUPW + wl
                n0 = wv * WIN
                cnt = min(WIN, NPC - n0)
                nc.sync.dma_start(out=y_t[n0:n0 + cnt, :],
                                  in_=ysb[:cnt, wl * HD:(wl + 1) * HD])

    nc.compile()
    return nc


def run_numpy_model(cfg, x, W, attn_l, attn_r, bias, fc_w, fc_b, src, dst):
    """Numpy model of the kernel math (for validation)."""
    feat = np.einsum("ni,hio->nho", x, W)
    el = np.einsum("nho,ho->nh", feat, attn_l)
    er = np.einsum("nho,ho->nh", feat, attn_r)
    e = el[src] + er[dst]
    e = np.where(e > 0, e, NEG_SLOPE * e)
    w = np.exp(e)
    s = np.zeros((cfg.N, cfg.H), np.float32)
    np.add.at(s, dst, w)
    g = np.einsum("nho,od->nhd", feat, fc_w)
    usum = np.zeros((cfg.N, cfg.H, cfg.D), np.float32)
    np.add.at(usum, dst, w[:, :, None] * g[src])
    out = usum / np.maximum(s, 1e-30)[:, :, None]
    return out + (bias @ fc_w)[None] + fc_b[None, None, :]


def make_in_maps(cfg, inputs, A_w, S_list, ST_list):
    import ml_dtypes
    BF = np.dtype(ml_dtypes.bfloat16)
    x = np.asarray(inputs["x"], np.float32)
    xT, WT, wcat, bias_flat, fcb_tiled = host_layouts(
        cfg, x, inputs["W"], inputs["attn_l"], inputs["attn_r"],
        inputs["bias"], inputs["fc_w"], inputs["fc_b"])
    xTb = np.ascontiguousarray(xT.astype(BF))
    in_maps = []
    for c in range(cfg.NCORES):
        in_maps.append({
            "xT": xTb,
            "xTown": np.ascontiguousarray(xTb[:, c * cfg.NPC:(c + 1) * cfg.NPC]),
            "WT": WT, "wcat": wcat, "bias_flat": bias_flat,
            "fcb_tiled": fcb_tiled,
            "A_idx": A_w[c], "S_oh": S_list[c], "ST_oh": ST_list[c],
        })
    return in_maps


# ----------------------------------------------------------------------------
# Self-contained entry point: full inputs in, full output out.
# ----------------------------------------------------------------------------

def kernel(**inputs):
    import numpy as np
    from concourse import bass_utils

    cfg = Cfg()
    src = np.asarray(inputs["src"])
    dst = np.asarray(inputs["dst"])
    assert src.shape == (cfg.E,) and dst.shape == (cfg.E,)
    st, A_w, S_list, ST_list = preprocess(cfg, src, dst)
    nc = build_program(cfg, st)
    in_maps = make_in_maps(cfg, inputs, A_w, S_list, ST_list)
    res = bass_utils.run_bass_kernel_spmd(
        nc, in_maps, core_ids=list(range(cfg.NCORES)))
    y = np.concatenate([r["y"] for r in res.results], axis=0)
    return np.ascontiguousarray(y.reshape(cfg.N, cfg.H, cfg.D).astype(np.float32))


# revision 6
# speedup vs baseline: 2.0893x; 1.0344x over previous
"""GAT (graph attention) Bass kernel for TRN2, 8-core SPMD — v2.

Math (equivalent to the reference up to fp reassociation):
  feat = x @ W (per head);  el/er = feat . attn_l/attn_r  ==>  el = x @ wl, er = x @ wr
  g    = feat @ fc_w (per head)                           ==>  g  = x @ WFC
  w_e  = exp(leakyrelu(el[src] + er[dst]))       (softmax without max-subtraction)
  s[d] = sum_{e->d} w_e ;  usum[d] = sum_{e->d} w_e * g[src]
  y[d] = usum[d]/s[d] + bias@fc_w + fc_b

Sharding: dst-range partitioning. Core k owns nodes [k*NPC, (k+1)*NPC).
Each core (replicated) computes the node row table [el|g] for all N nodes
(bf16 matmuls, f32 rows), gathers src rows per edge with dma_gather
(256B rows, the only per-edge DMA), and scatter-adds via one-hot matmuls
into PSUM window accumulators.

v2 vs v1:
  - er[dst] per edge comes from an SBUF-resident er table via a one-hot
    matmul (lhsT=ST) instead of a second dma_gather  -> halves GpSimd time.
  - The per-tile one-hot matrices S (scatter, [edge,slot]) and ST (er
    gather, [slot,edge]) are precomputed on the host as exact bf16 0/1
    matrices and DMA'd in -> removes the per-tile IS_EQ build from DVE.
  - Edge stream is ordered half-0-calls-first so gathers of src-half 0
    overlap the phase-1 build of src-half 1 (row table split into two
    DRAM tensors for independent dependence tracking). All NG=5 group
    accumulators stay open in PSUM simultaneously.
  - Phase-1 projection matmuls run in bf16 (fp32 PE matmuls take 2 passes).

Edges (host-side index prep only) are bucketed by (core, window, src-half)
— the src-half split keeps gather indices < 32768 (int16 limit).
"""

import numpy as np

import concourse.bass as bass
import concourse.mybir as mybir
import concourse.tile as tile
from concourse import bacc

F32 = mybir.dt.float32
BF16 = mybir.dt.bfloat16
FP8 = mybir.dt.float8e4
I16 = mybir.dt.int16
ALU = mybir.AluOpType
ACTF = mybir.ActivationFunctionType

NEG_SLOPE = 0.2


class Cfg:
    def __init__(self, N=50000, E=1200000, CT=31, GROUPW=11, DMA_SCRATCH=16384):
        self.N = N
        self.E = E
        self.IN = 256           # input feature dim
        self.H = 4              # heads
        self.O = 64             # per-head out dim
        self.D = 10             # final fc dim
        self.NCORES = 8
        self.WIN = 128          # dst nodes per window
        self.GROUPW = GROUPW    # windows per PSUM bank (44*11=484 <= 512 f32)
        self.CT = CT            # tiles per gather call
        self.DMA_SCRATCH = DMA_SCRATCH
        assert CT * 8 + 1 <= 256   # SWDGE ring: descs per lane
        assert N % self.NCORES == 0
        self.NPC = N // self.NCORES
        self.NW = -(-self.NPC // self.WIN)
        self.NG = -(-self.NW // GROUPW)
        assert N % 2 == 0
        self.SPLIT = N // 2
        assert self.SPLIT <= 32767 and self.NPC <= 32767
        self.KI = -(-self.IN // 128)             # x chunks (2)
        self.KHO = -(-(self.H * self.O) // 128)  # ho chunks (2)
        self.AUXW = 2 * self.H + self.H * self.D  # 48: [er|el|g]
        self.ROWW = 64                            # table row width (f32) = 256B
        self.HD = self.H * self.D                 # 40
        self.MAINW = self.H + self.HD             # 44: [el|g] in main table


class Structure:
    """Compile-time stream structure shared by host packer and program builder.

    Stream order is half-major: all half-0 calls (groups 0..NG-1), then all
    half-1 calls.  Each group's PSUM accumulator is opened by its first
    half-0 tile and closed by its last half-1 tile; all NG accumulators are
    live simultaneously.
    """

    def __init__(self, cfg: Cfg, T_wh: np.ndarray):
        self.T_wh = T_wh  # [NW, 2] tiles per (window, half)
        tile_meta = []    # [w, half, first, last]
        call_meta = []    # (tile_start, ntiles, half)
        pos_wh = np.zeros((cfg.NW, 2), np.int64)  # first tile index of bucket
        group_tiles = [[] for _ in range(cfg.NG)]
        for half in (0, 1):
            for g in range(cfg.NG):
                ws = range(g * cfg.GROUPW, min((g + 1) * cfg.GROUPW, cfg.NW))
                run_start = len(tile_meta)
                for w in ws:
                    pos_wh[w, half] = len(tile_meta)
                    for t in range(T_wh[w, half]):
                        group_tiles[g].append(len(tile_meta))
                        tile_meta.append([w, half, False, False])
                run_len = len(tile_meta) - run_start
                o = 0
                while o < run_len:
                    c = min(cfg.CT, run_len - o)
                    call_meta.append((run_start + o, c, half))
                    o += c
        for g in range(cfg.NG):
            tile_meta[group_tiles[g][0]][2] = True    # first (an h0 tile)
            tile_meta[group_tiles[g][-1]][3] = True   # last (an h1 tile)
        c0, ct, hf = call_meta[-1]
        if ct > 6:
            call_meta[-1] = (c0, ct - 4, hf)
            call_meta.append((c0 + ct - 4, 4, hf))
        self.tile_meta = tile_meta
        self.call_meta = call_meta
        self.pos_wh = pos_wh
        self.NT = len(tile_meta)


def preprocess(cfg: Cfg, src: np.ndarray, dst: np.ndarray):
    """Host-side index/layout prep (no float arithmetic).

    Returns (structure, A_idx[NC,128,NT*8] i16, S[NC,128,NT*128] bf16,
    ST[NC,128,NT*128] bf16).  S/ST are exact 0/1 one-hot matrices.
    """
    OH_DT = np.dtype(mybir.dt.np(FP8))

    src = np.asarray(src, np.int64)
    dst = np.asarray(dst, np.int64)
    core = dst // cfg.NPC
    dstl = dst - core * cfg.NPC
    w = dstl // cfg.WIN
    off = dstl - w * cfg.WIN
    half = (src >= cfg.SPLIT).astype(np.int64)
    srcr = src - half * cfg.SPLIT

    key = (core * cfg.NW + w) * 2 + half
    order = np.argsort(key, kind="stable")
    nk = cfg.NCORES * cfg.NW * 2
    cnt = np.bincount(key, minlength=nk)
    starts = np.zeros(nk + 1, np.int64)
    np.cumsum(cnt, out=starts[1:])
    cnt_cwh = cnt.reshape(cfg.NCORES, cfg.NW, 2)
    T_wh = np.maximum(-(-cnt_cwh.max(axis=0) // cfg.WIN), 1)  # [NW,2]
    st = Structure(cfg, T_wh)

    NPOS = st.NT * 128
    A_idx = np.zeros((cfg.NCORES, NPOS), np.int16)
    offp = np.full((cfg.NCORES, NPOS), -1, np.int64)
    for c in range(cfg.NCORES):
        for wv in range(cfg.NW):
            for h in (0, 1):
                kk = (c * cfg.NW + wv) * 2 + h
                n = cnt[kk]
                if n == 0:
                    continue
                eids = order[starts[kk]:starts[kk] + n]
                p0 = st.pos_wh[wv, h] * 128
                A_idx[c, p0:p0 + n] = srcr[eids]
                offp[c, p0:p0 + n] = off[eids]

    def wrap16(a):  # [NPOS] -> [128, NPOS//16]
        return np.tile(np.ascontiguousarray(a.reshape(-1, 16).T), (8, 1))

    A_w = np.stack([wrap16(A_idx[c]) for c in range(cfg.NCORES)])

    pos = np.arange(NPOS, dtype=np.int64)
    S_list, ST_list = [], []
    for c in range(cfg.NCORES):
        o = offp[c]
        valid = o >= 0
        i = pos[valid]
        ov = o[valid]
        S = np.zeros((128, NPOS), OH_DT)
        S[i % 128, (i // 128) * 128 + ov] = 1
        ST = np.zeros((128, NPOS), OH_DT)
        ST[ov, i] = 1
        S_list.append(S)
        ST_list.append(ST)
    return st, A_w, S_list, ST_list


def host_layouts(cfg: Cfg, x, W, attn_l, attn_r, bias, fc_w, fc_b):
    """Pure layout transforms of the inputs (no arithmetic)."""
    H, O, D = cfg.H, cfg.O, cfg.D
    xT = np.ascontiguousarray(np.asarray(x, np.float32).T)          # [IN, N]
    WT = np.ascontiguousarray(
        np.asarray(W, np.float32).transpose(0, 2, 1).reshape(H * O, cfg.IN))
    wcat = np.zeros((H * O, cfg.AUXW), np.float32)                  # [ho, er|el|g]
    for h in range(H):
        r = slice(h * O, (h + 1) * O)
        wcat[r, h] = attn_r[h]
        wcat[r, H + h] = attn_l[h]
        wcat[r, 2 * H + h * D:2 * H + (h + 1) * D] = fc_w
    bias_flat = np.asarray(bias, np.float32).reshape(H * O, 1)
    fcb_tiled = np.tile(np.asarray(fc_b, np.float32), H).reshape(1, H * D)
    return xT, WT, wcat, bias_flat, fcb_tiled


def build_program(cfg: Cfg, st: Structure):
    nc = bacc.Bacc(trn_type="TRN2", num_swdge_queues=1,
                   dynamic_dma_scratch_size=cfg.DMA_SCRATCH)
    N, IN, H, O, D = cfg.N, cfg.IN, cfg.H, cfg.O, cfg.D
    KI, KHO, AUXW, ROWW, HD, MAINW = (cfg.KI, cfg.KHO, cfg.AUXW, cfg.ROWW,
                                      cfg.HD, cfg.MAINW)
    WIN, NW, NG, GROUPW, NPC, CT, SPLIT = (cfg.WIN, cfg.NW, cfg.NG, cfg.GROUPW,
                                           cfg.NPC, cfg.CT, cfg.SPLIT)
    NT = st.NT

    xT = nc.dram_tensor("xT", [IN, N], BF16, kind="ExternalInput")
    WTt = nc.dram_tensor("WT", [H * O, IN], F32, kind="ExternalInput")
    wcat_t = nc.dram_tensor("wcat", [H * O, AUXW], F32, kind="ExternalInput")
    bias_t = nc.dram_tensor("bias_flat", [H * O, 1], F32, kind="ExternalInput")
    fcb_t = nc.dram_tensor("fcb_tiled", [1, HD], F32, kind="ExternalInput")
    Aidx_t = nc.dram_tensor("A_idx", [128, NT * 8], I16, kind="ExternalInput")
    S_t = nc.dram_tensor("S_oh", [128, NT * 128], FP8, kind="ExternalInput")
    ST_t = nc.dram_tensor("ST_oh", [128, NT * 128], FP8, kind="ExternalInput")
    y_t = nc.dram_tensor("y", [NPC, HD], F32, kind="ExternalOutput")

    row_h = [nc.dram_tensor(f"row_h{h}", [SPLIT, ROWW], F32, kind="Internal")
             for h in (0, 1)]

    NB = 4  # node tiles per phase-1 load batch
    own0 = None  # filled per-core via own_base input? -- no: SPMD shared program

    # Per-core own range differs between cores, but the program is shared.
    # The er pass reads xT columns [own_base, own_base+NPC); own_base is
    # supplied via a 1-element index DMA... simpler: the er pass uses a
    # dram input holding the own x slice? That re-adds 6.4MB upload.
    # Instead supply own_base as a per-core DRAM slice of xT via a separate
    # ExternalInput xTo view prepared host-side without copying (numpy view).
    xTo = nc.dram_tensor("xTown", [IN, NPC], BF16, kind="ExternalInput")

    with tile.TileContext(nc) as tc, \
            tc.tile_pool(name="const", bufs=1) as cp, \
            tc.tile_pool(name="p1", bufs=3) as p1, \
            tc.tile_pool(name="p1ps", bufs=2, space="PSUM") as p1ps, \
            tc.tile_pool(name="gath", bufs=3) as gp, \
            tc.tile_pool(name="tp", bufs=4) as tp, \
            tc.tile_pool(name="erps", bufs=1, space="PSUM") as erp, \
            tc.tile_pool(name="acc", bufs=1, space="PSUM") as accp, \
            tc.tile_pool(name="outp", bufs=2) as op:

        # ---------- phase 0: constants ----------
        wt_sb = cp.tile([128, KHO, IN], F32)
        wcat_sb = cp.tile([128, KHO, AUXW], F32)
        bf_sb = cp.tile([128, KHO, 1], F32)
        for a in range(KHO):
            r = slice(a * 128, (a + 1) * 128)
            nc.sync.dma_start(out=wt_sb[:, a, :], in_=WTt[r, :])
            nc.sync.dma_start(out=wcat_sb[:, a, :], in_=wcat_t[r, :])
            nc.sync.dma_start(out=bf_sb[:, a, :], in_=bias_t[r, :])
        fcb_sb = cp.tile([1, HD], F32)
        nc.sync.dma_start(out=fcb_sb[:], in_=fcb_t[:])

        # aux = WT.T @ wcat : [IN(pad 256), AUXW] ; stored bf16 for phase 1
        auxb = cp.tile([128, KI, AUXW], BF16)
        for m in range(KI):
            aps = p1ps.tile([128, AUXW], F32, tag="rps")
            for k in range(KHO):
                nc.tensor.matmul(out=aps[:], lhsT=wt_sb[:, k, m * 128:(m + 1) * 128],
                                 rhs=wcat_sb[:, k, :], start=(k == 0), stop=(k == KHO - 1))
            nc.vector.tensor_copy(out=auxb[:, m, :], in_=aps[:])

        # bias@fc_w + fc_b, replicated to 128 partitions and GROUPW windows
        brow_ps = p1ps.tile([1, HD], F32, tag="rps")
        for k in range(KHO):
            nc.tensor.matmul(out=brow_ps[:], lhsT=bf_sb[:, k, :],
                             rhs=wcat_sb[:, k, 2 * H:AUXW],
                             start=(k == 0), stop=(k == KHO - 1))
        brow_sb = cp.tile([1, HD], F32)
        nc.vector.tensor_add(out=brow_sb[:], in0=brow_ps[:], in1=fcb_sb[:])
        ones_sb = cp.tile([1, 128], F32)
        nc.vector.memset(ones_sb[:], 1.0)
        brep_ps = p1ps.tile([128, HD], F32, tag="rps")
        nc.tensor.matmul(out=brep_ps[:], lhsT=ones_sb[:], rhs=brow_sb[:],
                         start=True, stop=True)
        brep_sb = cp.tile([128, GROUPW * HD], F32)
        for wl in range(GROUPW):
            nc.vector.tensor_copy(out=brep_sb[:, wl * HD:(wl + 1) * HD], in_=brep_ps[:])

        # ---------- phase 1a: er table for own dst range (SBUF-resident) ----
        er_all = cp.tile([128, NW, H], BF16)

        def er_pass():
            ntiles_er = -(-NPC // 128)
            for b in range(0, ntiles_er, NB):
                bt = min(NB, ntiles_er - b)
                n0 = b * 128
                bcnt = min(NB * 128, NPC - n0)
                xte = p1.tile([128, KI, NB * 128], BF16, tag="xte")
                for k in range(KI):
                    nc.sync.dma_start(out=xte[:, k, :bcnt],
                                      in_=xTo[k * 128:(k + 1) * 128, n0:n0 + bcnt])
                for j in range(bt):
                    cnt = min(128, NPC - (b + j) * 128)
                    rps = p1ps.tile([128, H], F32, tag="rps")
                    for k in range(KI):
                        nc.tensor.matmul(
                            out=rps[:cnt, :],
                            lhsT=xte[:, k, j * 128:j * 128 + cnt],
                            rhs=auxb[:, k, 0:H],
                            start=(k == 0), stop=(k == KI - 1))
                    nc.vector.tensor_copy(out=er_all[:cnt, b + j, :], in_=rps[:cnt, :])

        # ---------- phase 1b: main row tables [el|g], one per src half ------
        ntiles_half = -(-SPLIT // 128)

        def emit_half_batch(hf, b):
            col0 = hf * SPLIT
            ntiles = ntiles_half
            if True:
                bt = min(NB, ntiles - b)
                n0 = b * 128
                bcnt = min(NB * 128, SPLIT - n0)
                xt = p1.tile([128, KI, NB * 128], BF16, tag=f"xt{hf}")
                for k in range(KI):
                    nc.sync.dma_start(
                        out=xt[:, k, :bcnt],
                        in_=xT[k * 128:(k + 1) * 128, col0 + n0:col0 + n0 + bcnt])
                rsb = p1.tile([128, NB, ROWW], F32, tag=f"rsb{hf}")
                for j in range(bt):
                    cnt = min(128, SPLIT - (b + j) * 128)
                    rps = p1ps.tile([128, MAINW], F32, tag="rps")
                    for k in range(KI):
                        nc.tensor.matmul(
                            out=rps[:cnt, :],
                            lhsT=xt[:, k, j * 128:j * 128 + cnt],
                            rhs=auxb[:, k, H:AUXW],
                            start=(k == 0), stop=(k == KI - 1))
                    nc.vector.tensor_copy(out=rsb[:cnt, j, :MAINW], in_=rps[:cnt, :])
                if bcnt == bt * 128:
                    out_ap = row_h[hf][n0:n0 + bt * 128, :].rearrange(
                        "(j p) c -> p j c", p=128)
                    nc.sync.dma_start(out=out_ap, in_=rsb[:, :bt, :])
                else:  # partial final tile: per-tile writes
                    for j in range(bt):
                        cnt = min(128, SPLIT - (b + j) * 128)
                        nc.sync.dma_start(
                            out=row_h[hf][(b + j) * 128:(b + j) * 128 + cnt, :],
                            in_=rsb[:cnt, j, :])

        # warm up the Q7 DMAGather library during the head
        wu_tab = nc.dram_tensor("wu_tab", [128, ROWW], F32, kind="Internal")
        wu_idx = cp.tile([128, 8], I16)
        nc.vector.memset(wu_idx[:], 0)
        wu_out = cp.tile([128, 1, ROWW], F32)
        nc.gpsimd.dma_gather(wu_out[:, :1, :], wu_tab[:, :], wu_idx[:, :8],
                             128, nc.gpsimd.to_reg(128), ROWW, queue_num=0,
                             single_packet=False)

        er_pass()
        for b in range(0, ntiles_half, NB):
            emit_half_batch(0, b)
        h1_batches = list(range(0, ntiles_half, NB))
        h1_next = [0]

        def emit_some_h1(k):
            while k > 0 and h1_next[0] < len(h1_batches):
                emit_half_batch(1, h1_batches[h1_next[0]])
                h1_next[0] += 1
                k -= 1

        # ---------- phase 2: edge stream ----------
        gtiles = {}

        def get_gps(g):
            if g not in gtiles:
                gtiles[g] = accp.tile([128, GROUPW * MAINW], F32,
                                      tag=f"gps{g}", name=f"gps{g}")
            return gtiles[g]

        nreg_cache = {}

        def nreg(n):
            if n not in nreg_cache:
                nreg_cache[n] = nc.gpsimd.to_reg(n)
            return nreg_cache[n]

        for ci, (c0, ctiles, half) in enumerate(st.call_meta):
            if half == 1:
                emit_some_h1(len(h1_batches))  # flush any remainder
            aidx = gp.tile([128, CT * 8], I16, tag="aidx", bufs=3)
            nc.sync.dma_start(out=aidx[:, :ctiles * 8],
                              in_=Aidx_t[:, c0 * 8:(c0 + ctiles) * 8])
            S_sb = gp.tile([128, CT, WIN], FP8, tag="S_sb", bufs=3)
            nc.sync.dma_start(out=S_sb[:, :ctiles, :],
                              in_=S_t[:, c0 * 128:(c0 + ctiles) * 128])
            ST_sb = gp.tile([128, CT, WIN], FP8, tag="ST_sb", bufs=3)
            nc.sync.dma_start(out=ST_sb[:, :ctiles, :],
                              in_=ST_t[:, c0 * 128:(c0 + ctiles) * 128])
            abuf = gp.tile([128, CT, ROWW], F32, tag="abuf", bufs=4)
            nc.gpsimd.dma_gather(abuf[:, :ctiles, :], row_h[half][:, :],
                                 aidx[:, :ctiles * 8],
                                 ctiles * 128, nreg(ctiles * 128), ROWW, queue_num=0,
                                 single_packet=False)

            # er per edge: one-hot gather matmul from SBUF er table
            er_ps = erp.tile([128, CT * H], F32, tag="erps")
            for j in range(ctiles):
                wv = st.tile_meta[c0 + j][0]
                nc.tensor.matmul(out=er_ps[:, j * H:(j + 1) * H],
                                 lhsT=ST_sb[:, j, :], rhs=er_all[:, wv, :],
                                 start=True, stop=True)

            ne = ctiles * H
            esb = tp.tile([128, CT * H], F32, tag="esb")
            nc.vector.tensor_tensor(
                out=esb[:, :ne].rearrange("p (t h) -> p t h", h=H),
                in0=abuf[:, :ctiles, 0:H],
                in1=er_ps[:, :ne].rearrange("p (t h) -> p t h", h=H),
                op=ALU.add)
            nc.vector.scalar_tensor_tensor(
                out=esb[:, :ne], in0=esb[:, :ne], scalar=NEG_SLOPE,
                in1=esb[:, :ne], op0=ALU.mult, op1=ALU.max)
            nc.scalar.activation(out=esb[:, :ne], in_=esb[:, :ne], func=ACTF.Exp)
            # rhs chunk tile: per tile j, cols [0:H]=w (bf16), [H:MAINW]=w*g
            mgc = tp.tile([128, CT, MAINW], BF16, tag="mgc")
            nc.vector.tensor_copy(
                out=mgc[:, :ctiles, 0:H], in_=esb[:, :ne].rearrange(
                    "p (t h) -> p t h", h=H))
            nc.vector.tensor_tensor(
                out=mgc[:, :ctiles, H:MAINW].rearrange("p t (h d) -> p t h d", h=H),
                in0=abuf[:, :ctiles, H:MAINW].rearrange("p t (h d) -> p t h d", h=H),
                in1=esb[:, :ne].rearrange("p (t h) -> p t h", h=H)
                    .to_broadcast([128, ctiles, H, D]),
                op=ALU.mult)

            for j in range(ctiles):
                wv, half_, first, last = st.tile_meta[c0 + j]
                g = wv // GROUPW
                gps = get_gps(g)
                wloc = wv - g * GROUPW
                base = wloc * MAINW
                nc.tensor.matmul(out=gps[:, base:base + MAINW],
                                 lhsT=S_sb[:, j, :], rhs=mgc[:, j, :],
                                 start=first, stop=last)
            if half == 0 and ci >= 2:
                emit_some_h1(3)

        # ---------- phase 3: normalize + output ----------
        for g in range(NG):
            gps = gtiles[g]
            glen = min(GROUPW, NW - g * GROUPW)
            gv = gps[:].rearrange("p (w c) -> p w c", c=MAINW)
            sg = op.tile([128, GROUPW * H], F32, tag="sg")
            nc.vector.tensor_scalar_max(out=sg[:, :glen * H], in0=gv[:, :glen, 0:H],
                                        scalar1=1e-30)
            rs = op.tile([128, GROUPW * H], F32, tag="rs")
            nc.vector.reciprocal(out=rs[:, :glen * H], in_=sg[:, :glen * H])
            ysb = op.tile([128, GROUPW * HD], F32, tag="ysb")
            nc.vector.tensor_tensor(
                out=ysb[:, :glen * HD].rearrange("p (w h d) -> p w h d", h=H, d=D),
                in0=gv[:, :glen, H:MAINW].rearrange("p w (h d) -> p w h d", h=H),
                in1=rs[:, :glen * H].rearrange("p (w h) -> p w h", h=H)
                    .to_broadcast([128, glen, H, D]),
                op=ALU.mult)
            nc.vector.tensor_add(out=ysb[:, :glen * HD], in0=ysb[:, :glen * HD],
                                 in1=brep_sb[:, :glen * HD])
            for wl in range(glen):
                wv = g * GROUPW + wl
                n0 = wv * WIN
                cnt = min(WIN, NPC - n0)
                nc.sync.dma_start(out=y_t[n0:n0 + cnt, :],
                                  in_=ysb[:cnt, wl * HD:(wl + 1) * HD])

    nc.compile()
    return nc


def run_numpy_model(cfg, x, W, attn_l, attn_r, bias, fc_w, fc_b, src, dst):
    """Numpy model of the kernel math (for validation)."""
    feat = np.einsum("ni,hio->nho", x, W)
    el = np.einsum("nho,ho->nh", feat, attn_l)
    er = np.einsum("nho,ho->nh", feat, attn_r)
    e = el[src] + er[dst]
    e = np.where(e > 0, e, NEG_SLOPE * e)
    w = np.exp(e)
    s = np.zeros((cfg.N, cfg.H), np.float32)
    np.add.at(s, dst, w)
    g = np.einsum("nho,od->nhd", feat, fc_w)
    usum = np.zeros((cfg.N, cfg.H, cfg.D), np.float32)
    np.add.at(usum, dst, w[:, :, None] * g[src])
    out = usum / np.maximum(s, 1e-30)[:, :, None]
    return out + (bias @ fc_w)[None] + fc_b[None, None, :]


def make_in_maps(cfg, inputs, A_w, S_list, ST_list):
    import ml_dtypes
    BF = np.dtype(ml_dtypes.bfloat16)
    x = np.asarray(inputs["x"], np.float32)
    xT, WT, wcat, bias_flat, fcb_tiled = host_layouts(
        cfg, x, inputs["W"], inputs["attn_l"], inputs["attn_r"],
        inputs["bias"], inputs["fc_w"], inputs["fc_b"])
    xTb = np.ascontiguousarray(xT.astype(BF))
    in_maps = []
    for c in range(cfg.NCORES):
        in_maps.append({
            "xT": xTb,
            "xTown": np.ascontiguousarray(xTb[:, c * cfg.NPC:(c + 1) * cfg.NPC]),
            "WT": WT, "wcat": wcat, "bias_flat": bias_flat,
            "fcb_tiled": fcb_tiled,
            "A_idx": A_w[c], "S_oh": S_list[c], "ST_oh": ST_list[c],
        })
    return in_maps


# ----------------------------------------------------------------------------
# Self-contained entry point: full inputs in, full output out.
# ----------------------------------------------------------------------------

def kernel(**inputs):
    import numpy as np
    from concourse import bass_utils

    cfg = Cfg()
    src = np.asarray(inputs["src"])
    dst = np.asarray(inputs["dst"])
    assert src.shape == (cfg.E,) and dst.shape == (cfg.E,)
    st, A_w, S_list, ST_list = preprocess(cfg, src, dst)
    nc = build_program(cfg, st)
    in_maps = make_in_maps(cfg, inputs, A_w, S_list, ST_list)
    res = bass_utils.run_bass_kernel_spmd(
        nc, in_maps, core_ids=list(range(cfg.NCORES)))
    y = np.concatenate([r["y"] for r in res.results], axis=0)
    return np.ascontiguousarray(y.reshape(cfg.N, cfg.H, cfg.D).astype(np.float32))
